# revision 8
# baseline (speedup 1.0000x reference)
"""Trainium2 Bass kernel for nn_Decoder (ragged GRU + MLP decoder).

Strategy
--------
Data-parallel over 8 NeuronCores.  Rows are bucketed by n (sequence
length) and distributed so that every core receives an IDENTICAL
multiset of n-values -> one static SPMD program works for all cores.
Rows with n == 0 never touch the device (output is zero).  A remainder
of < 120 rows (to make the per-value counts divisible by 8) is computed
on the host in numpy.

Per core, rows are sorted by n descending and grouped into "pairs" of
1024 rows.  The GRU state lives transposed in SBUF as [128, 512] bf16
tiles: partitions 0-63 hold h for rows [base, base+512), partitions
64-127 hold h for rows [base+512, base+1024).  Gate matmuls use
block-diagonal [128,128] bf16 weights so each gate for BOTH halves is
produced in a single N=512 matmul with the gate value for every row on
a distinct partition -> all elementwise ops run with 128 busy lanes.

Per pair only max(n in pair) GRU steps run (ragged skip).  Valid output
cells are written via exact-extent DMAs; everything else relies on the
PJRT path zero-initializing ExternalOutput buffers.

GRU algebra (x==h after step 0, PyTorch gate convention):
    r = sigmoid(Wr h + br)        Wr = (W_ih + W_hh)[0:64],   br = (b_ih+b_hh)[0:64]
    u = sigmoid(Wu h + bu)        Wu = (W_ih + W_hh)[64:128], bu = (b_ih+b_hh)[64:128]
    c = tanh(W_ic h + b_ic + r*(W_hc h + b_hc))
    h' = c + u*(h - c)
Step 0 uses x=0, h=z: W* = W_hh blocks only, same biases, ic = b_ic.
"""

import os
import sys

import numpy as np

sys.path.insert(0, "/opt/trn_rl_repo")

import ml_dtypes  # noqa: E402

BF16 = ml_dtypes.bfloat16

from concourse import bacc, bass, tile  # noqa: E402
from concourse.bass_utils import run_bass_kernel_spmd  # noqa: E402

mybir = bass.mybir
F32 = mybir.dt.float32
BF = mybir.dt.bfloat16
ALU = mybir.AluOpType
ACTF = mybir.ActivationFunctionType

B, H, D, MAXN, NCORES = 32768, 64, 128, 16, 8
PAIR = 1024          # rows per pair-group (2 partition halves x 512 cols)
COLS = PAIR // 2     # free-dim width of one pair tile

LAST_RESULTS = None  # BassKernelResults of the most recent run (for test.py)

_PROGRAM_CACHE = {}


def _blkdiag(a):
    out = np.zeros((128, 128), np.float32)
    out[:64, :64] = a
    out[64:, 64:] = a
    return out


def _build_program(S_pairs, V, B_pad):
    """Build the SPMD Bass program for the static schedule.

    S_pairs: steps to run for each pair-group (monotone non-increasing).
    V[t]:    number of valid rows at step t (same on every core).
    """
    npairs = len(S_pairs)
    nc = bacc.Bacc(None, target_bir_lowering=False)

    # ---- I/O ----------------------------------------------------------
    zt_d = nc.dram_tensor("zt", [128, npairs * COLS], BF, kind="ExternalInput")
    w_names = ["w_r0", "w_u0", "w_hc0", "w_r", "w_u", "w_ic", "w_hc",
               "w_i", "w_h1"]
    w_d = {k: nc.dram_tensor(k, [128, 128], BF, kind="ExternalInput")
           for k in w_names}
    w2_d = nc.dram_tensor("w2", [128, 128], BF, kind="ExternalInput")
    ones_d = nc.dram_tensor("ones1", [1, 128], F32, kind="ExternalInput")
    b2rep_d = nc.dram_tensor("b2rep", [1, 512], F32, kind="ExternalInput")
    bias_names = ["b_r", "b_u", "b_ic", "b_hc", "b_1"]
    bias_d = {k: nc.dram_tensor(k, [128, 1], F32, kind="ExternalInput")
              for k in bias_names}
    out_d = nc.dram_tensor("out", [B_pad, MAXN, D], F32, kind="ExternalOutput")

    with tile.TileContext(nc) as tc:
        with (
            tc.tile_pool(name="const", bufs=1) as cpool,
            tc.tile_pool(name="state", bufs=1) as spool,
            tc.tile_pool(name="work", bufs=3) as wpool,
            tc.tile_pool(name="outsb", bufs=2) as opool,
            tc.tile_pool(name="psum", bufs=1, space="PSUM") as ppool,
        )\
        :
            # ---- constants ------------------------------------------
            zt = cpool.tile([128, npairs * COLS], BF, name="zt_sb")
            nc.sync.dma_start(out=zt[:], in_=zt_d[:])
            w = {}
            for k in w_names:
                w[k] = cpool.tile([128, 128], BF, name=f"{k}_sb")
                nc.sync.dma_start(out=w[k][:], in_=w_d[k][:])
            w2 = cpool.tile([128, 128], BF, name="w2_sb")
            nc.sync.dma_start(out=w2[:], in_=w2_d[:])
            ones1 = cpool.tile([1, 128], F32, name="ones1_sb")
            nc.sync.dma_start(out=ones1[:], in_=ones_d[:])
            b2rep = cpool.tile([1, 512], F32, name="b2rep_sb")
            nc.sync.dma_start(out=b2rep[:], in_=b2rep_d[:])
            bias = {}
            for k in bias_names:
                bias[k] = cpool.tile([128, 1], F32, name=f"{k}_sb")
                nc.sync.dma_start(out=bias[k][:], in_=bias_d[k][:])

            hstate = spool.tile([128, npairs * COLS], BF, name="hstate")

            # ---- main loops -----------------------------------------
            for p in range(npairs):
                steps = S_pairs[p]
                hs = hstate[:, p * COLS:(p + 1) * COLS]
                for t in range(steps):
                    hin = zt[:, p * COLS:(p + 1) * COLS] if t == 0 else hs

                    # gate matmuls (bf16, N=512 each)
                    ru = ppool.tile([128, 2 * COLS], F32, tag="ru")
                    ichc = ppool.tile([128, 2 * COLS], F32, tag="ichc")
                    nc.tensor.matmul(ru[:, 0:COLS],
                                     w["w_r0" if t == 0 else "w_r"][:],
                                     hin)
                    nc.tensor.matmul(ru[:, COLS:2 * COLS],
                                     w["w_u0" if t == 0 else "w_u"][:],
                                     hin)
                    if t > 0:
                        nc.tensor.matmul(ichc[:, 0:COLS], w["w_ic"][:], hin,
                                         start=True, stop=False)
                    nc.tensor.matmul(ichc[:, COLS:2 * COLS],
                                     w["w_hc0" if t == 0 else "w_hc"][:],
                                     hin)

                    r_sb = wpool.tile([128, COLS], BF, tag="r")
                    u_sb = wpool.tile([128, COLS], BF, tag="u")
                    nc.scalar.activation(r_sb[:], ru[:, 0:COLS], ACTF.Sigmoid,
                                         bias=bias["b_r"][:])
                    nc.scalar.activation(u_sb[:], ru[:, COLS:2 * COLS],
                                         ACTF.Sigmoid, bias=bias["b_u"][:])

                    # rh = (hc_pre + b_hc) * r   (fp32 psum in0 -> 1x mode)
                    rh_sb = wpool.tile([128, COLS], BF, tag="rh")
                    nc.vector.scalar_tensor_tensor(
                        rh_sb[:], ichc[:, COLS:2 * COLS], bias["b_hc"][:],
                        r_sb[:], ALU.add, ALU.mult)

                    # cpre (ic half of ichc) += I @ rh   on the PE
                    nc.tensor.matmul(ichc[:, 0:COLS], w["w_i"][:], rh_sb[:],
                                     start=(t == 0), stop=True)

                    c_sb = wpool.tile([128, COLS], BF, tag="c")
                    nc.scalar.activation(c_sb[:], ichc[:, 0:COLS], ACTF.Tanh,
                                         bias=bias["b_ic"][:])

                    # h' = c + u*(h-c)
                    e_sb = wpool.tile([128, COLS], BF, tag="e")
                    f_sb = wpool.tile([128, COLS], BF, tag="f")
                    nc.vector.tensor_tensor(e_sb[:], hin, c_sb[:],
                                            ALU.subtract)
                    nc.gpsimd.tensor_tensor(f_sb[:], u_sb[:], e_sb[:],
                                            ALU.mult)
                    nc.vector.tensor_tensor(hs, c_sb[:], f_sb[:], ALU.add)

                    # ---- MLP + output for this (pair, step) ----------
                    h1p = ppool.tile([128, COLS], F32, tag="h1p")
                    nc.tensor.matmul(h1p[:], w["w_h1"][:], hs)
                    h1 = wpool.tile([128, COLS], BF, tag="h1")
                    nc.vector.tensor_scalar(h1[:], h1p[:], bias["b_1"][:],
                                            0.0, ALU.add, ALU.max)

                    outp = ppool.tile([128, 2 * COLS], F32, tag="outp")
                    for half in range(2):
                        po = half * COLS
                        nc.tensor.matmul(outp[:, po:po + COLS], ones1[:],
                                         b2rep[:], start=True, stop=False,
                                         skip_group_check=True)
                        for ck in range(4):
                            nc.tensor.matmul(
                                outp[:, po + ck * 128:po + (ck + 1) * 128],
                                h1[half * 64:(half + 1) * 64,
                                   ck * 128:(ck + 1) * 128],
                                w2[half * 64:(half + 1) * 64, :],
                                start=False, stop=True,
                                skip_group_check=True)

                    osb = opool.tile([128, 2 * COLS], F32, tag="osb")
                    nc.vector.tensor_copy(osb[:], outp[:])

                    for half in range(2):
                        for ck in range(4):
                            base = p * PAIR + half * COLS + ck * 128
                            vr = min(max(V[t] - base, 0), 128)
                            if vr > 0:
                                nc.sync.dma_start(
                                    out=out_d[base:base + vr, t, :],
                                    in_=osb[0:vr,
                                            half * COLS + ck * 128:
                                            half * COLS + (ck + 1) * 128])
    nc.compile()
    return nc


# ---------------------------------------------------------------------
# host-side numpy GRU+MLP for the remainder rows
def _host_rows(z_r, n_r, W_ih, W_hh, b_ih, b_hh, W1, b1, W2, b2):
    R = z_r.shape[0]
    out = np.zeros((R, MAXN, D), np.float32)
    if R == 0:
        return out

    def sig(v):
        return 1.0 / (1.0 + np.exp(-v))

    x = np.zeros_like(z_r)
    h = z_r.copy()
    for t in range(MAXN):
        gi = x @ W_ih.T + b_ih
        gh = h @ W_hh.T + b_hh
        r = sig(gi[:, :64] + gh[:, :64])
        u = sig(gi[:, 64:128] + gh[:, 64:128])
        c = np.tanh(gi[:, 128:] + r * gh[:, 128:])
        h = (1.0 - u) * c + u * h
        x = h
        out[:, t, :] = np.maximum(h @ W1 + b1, 0.0) @ W2 + b2
    out *= (np.arange(MAXN)[None, :, None] < n_r[:, None, None])
    return out


def kernel(z, n, W_ih, W_hh, b_ih, b_hh, W1, b1, W2, b2):
    global LAST_RESULTS
    z = np.asarray(z, np.float32)
    n = np.asarray(n, np.int32)
    W_ih = np.asarray(W_ih, np.float32)
    W_hh = np.asarray(W_hh, np.float32)
    b_ih = np.asarray(b_ih, np.float32)
    b_hh = np.asarray(b_hh, np.float32)
    W1 = np.asarray(W1, np.float32)
    b1 = np.asarray(b1, np.float32)
    W2 = np.asarray(W2, np.float32)
    b2 = np.asarray(b2, np.float32)

    # ---- partition rows across cores with identical n-multisets ------
    core_rows = [[] for _ in range(NCORES)]
    leftover = []
    n_core_vals = []
    for v in range(MAXN - 1, 0, -1):
        idx = np.where(n == v)[0]
        k = len(idx) // NCORES
        for c in range(NCORES):
            core_rows[c].append(idx[c * k:(c + 1) * k])
        leftover.append(idx[NCORES * k:])
        n_core_vals.append(np.full(k, v, np.int32))
    core_rows = [np.concatenate(cr) for cr in core_rows]
    leftover = np.concatenate(leftover)
    n_core = np.concatenate(n_core_vals) if n_core_vals else np.zeros(0, np.int32)

    B_real = len(core_rows[0])
    B_pad = max(((B_real + PAIR - 1) // PAIR) * PAIR, PAIR)
    n_sched = np.zeros(B_pad, np.int32)
    n_sched[:B_real] = n_core
    npairs = B_pad // PAIR
    S_pairs = tuple(int(n_sched[p * PAIR]) for p in range(npairs))
    V = tuple(int((n_sched > t).sum()) for t in range(MAXN))

    # ---- weights / schedule -> device constants ----------------------
    Wr = (W_ih[0:64] + W_hh[0:64]).T
    Wu = (W_ih[64:128] + W_hh[64:128]).T
    Wic = W_ih[128:192].T
    Whc = W_hh[128:192].T
    consts = {
        "w_r0": _blkdiag(W_hh[0:64].T),
        "w_u0": _blkdiag(W_hh[64:128].T),
        "w_hc0": _blkdiag(W_hh[128:192].T),
        "w_r": _blkdiag(Wr),
        "w_u": _blkdiag(Wu),
        "w_ic": _blkdiag(Wic),
        "w_hc": _blkdiag(Whc),
        "w_i": np.eye(128, dtype=np.float32),
        "w_h1": _blkdiag(W1),
    }
    consts = {k: v.astype(BF16) for k, v in consts.items()}
    consts["w2"] = np.vstack([W2, W2]).astype(BF16)
    consts["ones1"] = np.ones((1, 128), np.float32)
    consts["b2rep"] = np.tile(b2, 4)[None, :].astype(np.float32)
    consts["b_r"] = np.tile(b_ih[0:64] + b_hh[0:64], 2)[:, None].astype(np.float32)
    consts["b_u"] = np.tile(b_ih[64:128] + b_hh[64:128], 2)[:, None].astype(np.float32)
    consts["b_ic"] = np.tile(b_ih[128:192], 2)[:, None].astype(np.float32)
    consts["b_hc"] = np.tile(b_hh[128:192], 2)[:, None].astype(np.float32)
    consts["b_1"] = np.tile(b1, 2)[:, None].astype(np.float32)

    key = (S_pairs, V, B_pad)
    if key not in _PROGRAM_CACHE:
        _PROGRAM_CACHE.clear()
        _PROGRAM_CACHE[key] = _build_program(S_pairs, V, B_pad)
    nc = _PROGRAM_CACHE[key]

    in_maps = []
    for c in range(NCORES):
        zmat = np.zeros((B_pad, H), np.float32)
        zmat[:B_real] = z[core_rows[c]]
        zr = zmat.reshape(npairs, 2, COLS, H)
        ztc = zr.transpose(1, 3, 0, 2).reshape(128, npairs * COLS)
        m = dict(consts)
        m["zt"] = np.ascontiguousarray(ztc).astype(BF16)
        in_maps.append(m)

    res = run_bass_kernel_spmd(nc, in_maps, list(range(NCORES)))
    LAST_RESULTS = res

    # ---- gather ------------------------------------------------------
    x_out = np.zeros((B, MAXN, D), np.float32)
    for c in range(NCORES):
        x_out[core_rows[c]] = res.results[c]["out"][:B_real]
    if len(leftover):
        x_out[leftover] = _host_rows(z[leftover], n[leftover], W_ih, W_hh,
                                     b_ih, b_hh, W1, b1, W2, b2)
    mask = np.arange(MAXN)[None, :] < n[:, None]
    return x_out, mask


# revision 12
# speedup vs baseline: 1.3234x; 1.3234x over previous
"""Trainium2 Bass kernel for nn_Decoder (ragged GRU + MLP decoder).

Strategy
--------
Data-parallel over 8 NeuronCores.  Rows are bucketed by n (sequence
length) and distributed so that every core receives an IDENTICAL
multiset of n-values -> one static SPMD program works for all cores.
Rows with n == 0 never touch the device (output is zero).  A remainder
of < 120 rows (to make the per-value counts divisible by 8) is computed
on the host in numpy.

Per core, rows are sorted by n descending and grouped into "pairs" of
1024 rows.  The GRU state lives transposed in SBUF as [128, 512] bf16
tiles: partitions 0-63 hold h for rows [base, base+512), partitions
64-127 hold h for rows [base+512, base+1024).  Gate matmuls use
block-diagonal [128,128] bf16 weights so each gate for BOTH halves is
produced in a single N=512 matmul with the gate value for every row on
a distinct partition -> all elementwise ops run with 128 busy lanes.

Per pair only max(n in pair) GRU steps run (ragged skip).  Valid output
cells are written via exact-extent DMAs; everything else relies on the
PJRT path zero-initializing ExternalOutput buffers.

GRU algebra (x==h after step 0, PyTorch gate convention):
    r = sigmoid(Wr h + br)        Wr = (W_ih + W_hh)[0:64],   br = (b_ih+b_hh)[0:64]
    u = sigmoid(Wu h + bu)        Wu = (W_ih + W_hh)[64:128], bu = (b_ih+b_hh)[64:128]
    c = tanh(W_ic h + b_ic + r*(W_hc h + b_hc))
    h' = c + u*(h - c)
Step 0 uses x=0, h=z: W* = W_hh blocks only, same biases, ic = b_ic.
"""

import os
import sys

import numpy as np

sys.path.insert(0, "/opt/trn_rl_repo")

import ml_dtypes  # noqa: E402

BF16 = ml_dtypes.bfloat16

from concourse import bacc, bass, tile  # noqa: E402
from concourse.bass_utils import run_bass_kernel_spmd  # noqa: E402

mybir = bass.mybir
F32 = mybir.dt.float32
BF = mybir.dt.bfloat16
ALU = mybir.AluOpType
ACTF = mybir.ActivationFunctionType

B, H, D, MAXN, NCORES = 32768, 64, 128, 16, 8
PAIR = 1024          # rows per pair-group (2 partition halves x 512 cols)
COLS = PAIR // 2     # free-dim width of one pair tile

LAST_RESULTS = None  # BassKernelResults of the most recent run (for test.py)

_PROGRAM_CACHE = {}


def _blkdiag(a):
    out = np.zeros((128, 128), np.float32)
    out[:64, :64] = a
    out[64:, 64:] = a
    return out


def _build_program(S_pairs, V, B_pad):
    """Build the SPMD Bass program for the static schedule.

    S_pairs: steps to run for each pair-group (monotone non-increasing).
    V[t]:    number of valid rows at step t (same on every core).
    """
    npairs = len(S_pairs)
    nc = bacc.Bacc(None, target_bir_lowering=False)

    # ---- I/O ----------------------------------------------------------
    zt_d = nc.dram_tensor("zt", [128, npairs * COLS], BF, kind="ExternalInput")
    w_names = ["w_r0", "w_u0", "w_hc0", "w_r", "w_u", "w_ic", "w_hc",
               "w_i", "w_h1"]
    w_d = {k: nc.dram_tensor(k, [128, 128], BF, kind="ExternalInput")
           for k in w_names}
    w2_d = nc.dram_tensor("w2", [128, 128], BF, kind="ExternalInput")
    b2t_d = nc.dram_tensor("b2t", [128, 1024], F32, kind="ExternalInput")
    bias_names = ["b_r", "b_u", "b_ic", "b_hc", "b_1"]
    bias_d = {k: nc.dram_tensor(k, [128, 1], F32, kind="ExternalInput")
              for k in bias_names}
    out_d = nc.dram_tensor("out", [B_pad, MAXN, D], F32, kind="ExternalOutput")

    with tile.TileContext(nc) as tc:
        with (
            tc.tile_pool(name="const", bufs=1) as cpool,
            tc.tile_pool(name="state", bufs=1) as spool,
            tc.tile_pool(name="work", bufs=3) as wpool,
            tc.tile_pool(name="outsb", bufs=2) as opool,
            tc.tile_pool(name="psum", bufs=1, space="PSUM") as ppool,
        )\
        :
            # ---- constants ------------------------------------------
            zt = cpool.tile([128, npairs * COLS], BF, name="zt_sb")
            nc.sync.dma_start(out=zt[:], in_=zt_d[:])
            w = {}
            for k in w_names:
                w[k] = cpool.tile([128, 128], BF, name=f"{k}_sb")
                nc.sync.dma_start(out=w[k][:], in_=w_d[k][:])
            w2 = cpool.tile([128, 128], BF, name="w2_sb")
            nc.sync.dma_start(out=w2[:], in_=w2_d[:])
            b2t = cpool.tile([128, 1024], F32, name="b2t_sb")
            nc.sync.dma_start(out=b2t[:], in_=b2t_d[:])
            bias = {}
            for k in bias_names:
                bias[k] = cpool.tile([128, 1], F32, name=f"{k}_sb")
                nc.sync.dma_start(out=bias[k][:], in_=bias_d[k][:])

            hstate = spool.tile([128, npairs * COLS], BF, name="hstate")

            # ---- main loops -----------------------------------------
            for p in range(npairs):
                steps = S_pairs[p]
                hs = hstate[:, p * COLS:(p + 1) * COLS]
                for t in range(steps):
                    hin = zt[:, p * COLS:(p + 1) * COLS] if t == 0 else hs

                    # gate matmuls (bf16, N=512 each)
                    ru = ppool.tile([128, 2 * COLS], F32, tag="ru")
                    ichc = ppool.tile([128, 2 * COLS], F32, tag="ichc")
                    nc.tensor.matmul(ru[:, 0:COLS],
                                     w["w_r0" if t == 0 else "w_r"][:],
                                     hin)
                    nc.tensor.matmul(ru[:, COLS:2 * COLS],
                                     w["w_u0" if t == 0 else "w_u"][:],
                                     hin)
                    if t > 0:
                        nc.tensor.matmul(ichc[:, 0:COLS], w["w_ic"][:], hin,
                                         start=True, stop=False)
                    nc.tensor.matmul(ichc[:, COLS:2 * COLS],
                                     w["w_hc0" if t == 0 else "w_hc"][:],
                                     hin)

                    r_sb = wpool.tile([128, COLS], BF, tag="r")
                    u_sb = wpool.tile([128, COLS], BF, tag="u")
                    nc.scalar.activation(r_sb[:], ru[:, 0:COLS], ACTF.Sigmoid,
                                         bias=bias["b_r"][:])
                    nc.scalar.activation(u_sb[:], ru[:, COLS:2 * COLS],
                                         ACTF.Sigmoid, bias=bias["b_u"][:])

                    # rh = (hc_pre + b_hc) * r   (fp32 psum in0 -> 1x mode)
                    rh_sb = wpool.tile([128, COLS], BF, tag="rh")
                    nc.vector.scalar_tensor_tensor(
                        rh_sb[:], ichc[:, COLS:2 * COLS], bias["b_hc"][:],
                        r_sb[:], ALU.add, ALU.mult)

                    # cpre (ic half of ichc) += I @ rh   on the PE
                    nc.tensor.matmul(ichc[:, 0:COLS], w["w_i"][:], rh_sb[:],
                                     start=(t == 0), stop=True)

                    c_sb = wpool.tile([128, COLS], BF, tag="c")
                    nc.scalar.activation(c_sb[:], ichc[:, 0:COLS], ACTF.Tanh,
                                         bias=bias["b_ic"][:])

                    # h' = c + u*(h-c)
                    e_sb = wpool.tile([128, COLS], BF, tag="e")
                    f_sb = wpool.tile([128, COLS], BF, tag="f")
                    nc.vector.tensor_tensor(e_sb[:], hin, c_sb[:],
                                            ALU.subtract)
                    nc.gpsimd.tensor_tensor(f_sb[:], u_sb[:], e_sb[:],
                                            ALU.mult)
                    nc.vector.tensor_tensor(hs, c_sb[:], f_sb[:], ALU.add)

                    # ---- MLP + output for this (pair, step) ----------
                    h1p = ppool.tile([128, COLS], F32, tag="h1p")
                    nc.tensor.matmul(h1p[:], w["w_h1"][:], hs)
                    h1 = wpool.tile([128, COLS], BF, tag="h1")
                    nc.vector.tensor_scalar(h1[:], h1p[:], bias["b_1"][:],
                                            0.0, ALU.add, ALU.max)

                    outp = ppool.tile([128, 2 * COLS], F32, tag="outp")
                    for half in range(2):
                        po = half * COLS
                        for ck in range(4):
                            nc.tensor.matmul(
                                outp[:, po + ck * 128:po + (ck + 1) * 128],
                                h1[half * 64:(half + 1) * 64,
                                   ck * 128:(ck + 1) * 128],
                                w2[half * 64:(half + 1) * 64, :])

                    osb = opool.tile([128, 2 * COLS], F32, tag="osb")
                    nc.vector.tensor_tensor(osb[:], outp[:], b2t[:], ALU.add)

                    for half in range(2):
                        for ck in range(4):
                            base = p * PAIR + half * COLS + ck * 128
                            vr = min(max(V[t] - base, 0), 128)
                            if vr > 0:
                                nc.sync.dma_start(
                                    out=out_d[base:base + vr, t, :],
                                    in_=osb[0:vr,
                                            half * COLS + ck * 128:
                                            half * COLS + (ck + 1) * 128])
    nc.compile()
    return nc


# ---------------------------------------------------------------------
# host-side numpy GRU+MLP for the remainder rows
def _host_rows(z_r, n_r, W_ih, W_hh, b_ih, b_hh, W1, b1, W2, b2):
    R = z_r.shape[0]
    out = np.zeros((R, MAXN, D), np.float32)
    if R == 0:
        return out

    def sig(v):
        return 1.0 / (1.0 + np.exp(-v))

    x = np.zeros_like(z_r)
    h = z_r.copy()
    for t in range(MAXN):
        gi = x @ W_ih.T + b_ih
        gh = h @ W_hh.T + b_hh
        r = sig(gi[:, :64] + gh[:, :64])
        u = sig(gi[:, 64:128] + gh[:, 64:128])
        c = np.tanh(gi[:, 128:] + r * gh[:, 128:])
        h = (1.0 - u) * c + u * h
        x = h
        out[:, t, :] = np.maximum(h @ W1 + b1, 0.0) @ W2 + b2
    out *= (np.arange(MAXN)[None, :, None] < n_r[:, None, None])
    return out


def kernel(z, n, W_ih, W_hh, b_ih, b_hh, W1, b1, W2, b2):
    global LAST_RESULTS
    z = np.asarray(z, np.float32)
    n = np.asarray(n, np.int32)
    W_ih = np.asarray(W_ih, np.float32)
    W_hh = np.asarray(W_hh, np.float32)
    b_ih = np.asarray(b_ih, np.float32)
    b_hh = np.asarray(b_hh, np.float32)
    W1 = np.asarray(W1, np.float32)
    b1 = np.asarray(b1, np.float32)
    W2 = np.asarray(W2, np.float32)
    b2 = np.asarray(b2, np.float32)

    # ---- partition rows across cores with identical n-multisets ------
    core_rows = [[] for _ in range(NCORES)]
    leftover = []
    n_core_vals = []
    for v in range(MAXN - 1, 0, -1):
        idx = np.where(n == v)[0]
        k = len(idx) // NCORES
        for c in range(NCORES):
            core_rows[c].append(idx[c * k:(c + 1) * k])
        leftover.append(idx[NCORES * k:])
        n_core_vals.append(np.full(k, v, np.int32))
    core_rows = [np.concatenate(cr) for cr in core_rows]
    leftover = np.concatenate(leftover)
    n_core = np.concatenate(n_core_vals) if n_core_vals else np.zeros(0, np.int32)

    B_real = len(core_rows[0])
    B_pad = max(((B_real + PAIR - 1) // PAIR) * PAIR, PAIR)
    n_sched = np.zeros(B_pad, np.int32)
    n_sched[:B_real] = n_core
    npairs = B_pad // PAIR
    S_pairs = tuple(int(n_sched[p * PAIR]) for p in range(npairs))
    V = tuple(int((n_sched > t).sum()) for t in range(MAXN))

    # ---- weights / schedule -> device constants ----------------------
    Wr = (W_ih[0:64] + W_hh[0:64]).T
    Wu = (W_ih[64:128] + W_hh[64:128]).T
    Wic = W_ih[128:192].T
    Whc = W_hh[128:192].T
    consts = {
        "w_r0": _blkdiag(W_hh[0:64].T),
        "w_u0": _blkdiag(W_hh[64:128].T),
        "w_hc0": _blkdiag(W_hh[128:192].T),
        "w_r": _blkdiag(Wr),
        "w_u": _blkdiag(Wu),
        "w_ic": _blkdiag(Wic),
        "w_hc": _blkdiag(Whc),
        "w_i": np.eye(128, dtype=np.float32),
        "w_h1": _blkdiag(W1),
    }
    consts = {k: v.astype(BF16) for k, v in consts.items()}
    consts["w2"] = np.vstack([W2, W2]).astype(BF16)
    consts["b2t"] = np.tile(np.tile(b2, 8)[None, :], (128, 1)).astype(np.float32)
    consts["b_r"] = np.tile(b_ih[0:64] + b_hh[0:64], 2)[:, None].astype(np.float32)
    consts["b_u"] = np.tile(b_ih[64:128] + b_hh[64:128], 2)[:, None].astype(np.float32)
    consts["b_ic"] = np.tile(b_ih[128:192], 2)[:, None].astype(np.float32)
    consts["b_hc"] = np.tile(b_hh[128:192], 2)[:, None].astype(np.float32)
    consts["b_1"] = np.tile(b1, 2)[:, None].astype(np.float32)

    key = (S_pairs, V, B_pad)
    if key not in _PROGRAM_CACHE:
        _PROGRAM_CACHE.clear()
        _PROGRAM_CACHE[key] = _build_program(S_pairs, V, B_pad)
    nc = _PROGRAM_CACHE[key]

    in_maps = []
    for c in range(NCORES):
        zmat = np.zeros((B_pad, H), np.float32)
        zmat[:B_real] = z[core_rows[c]]
        zr = zmat.reshape(npairs, 2, COLS, H)
        ztc = zr.transpose(1, 3, 0, 2).reshape(128, npairs * COLS)
        m = dict(consts)
        m["zt"] = np.ascontiguousarray(ztc).astype(BF16)
        in_maps.append(m)

    res = run_bass_kernel_spmd(nc, in_maps, list(range(NCORES)))
    LAST_RESULTS = res

    # ---- gather ------------------------------------------------------
    x_out = np.zeros((B, MAXN, D), np.float32)
    for c in range(NCORES):
        x_out[core_rows[c]] = res.results[c]["out"][:B_real]
    if len(leftover):
        x_out[leftover] = _host_rows(z[leftover], n[leftover], W_ih, W_hh,
                                     b_ih, b_hh, W1, b1, W2, b2)
    mask = np.arange(MAXN)[None, :] < n[:, None]
    return x_out, mask


# revision 19
# speedup vs baseline: 1.6132x; 1.2190x over previous
"""Trainium2 Bass kernel for nn_Decoder (ragged GRU + MLP decoder).

Strategy
--------
Data-parallel over 8 NeuronCores.  Rows are bucketed by n (sequence
length) and distributed so that every core receives an IDENTICAL
multiset of n-values -> one static SPMD program works for all cores.
Rows with n == 0 never touch the device (output is zero).  A remainder
of < 120 rows (to make the per-value counts divisible by 8) is computed
on the host in numpy.

Per core, rows are sorted by n descending and grouped into "pairs" of
1024 rows.  The GRU state lives transposed in SBUF as [128, 512] bf16
tiles: partitions 0-63 hold h for rows [base, base+512), partitions
64-127 hold h for rows [base+512, base+1024).  Gate matmuls use
block-diagonal [128,128] bf16 weights so each gate for BOTH halves is
produced in a single N=512 matmul with the gate value for every row on
a distinct partition -> all elementwise ops run with 128 busy lanes.

Per pair only max(n in pair) GRU steps run (ragged skip).  Valid output
cells are written via exact-extent DMAs; everything else relies on the
PJRT path zero-initializing ExternalOutput buffers.

GRU algebra (x==h after step 0, PyTorch gate convention):
    r = sigmoid(Wr h + br)        Wr = (W_ih + W_hh)[0:64],   br = (b_ih+b_hh)[0:64]
    u = sigmoid(Wu h + bu)        Wu = (W_ih + W_hh)[64:128], bu = (b_ih+b_hh)[64:128]
    c = tanh(W_ic h + b_ic + r*(W_hc h + b_hc))
    h' = c + u*(h - c)
Step 0 uses x=0, h=z: W* = W_hh blocks only, same biases, ic = b_ic.
"""

import os
import sys

import numpy as np

sys.path.insert(0, "/opt/trn_rl_repo")

import ml_dtypes  # noqa: E402

BF16 = ml_dtypes.bfloat16

from concourse import bacc, bass, tile  # noqa: E402
from concourse.bass_utils import run_bass_kernel_spmd  # noqa: E402

mybir = bass.mybir
F32 = mybir.dt.float32
BF = mybir.dt.bfloat16
ALU = mybir.AluOpType
ACTF = mybir.ActivationFunctionType

B, H, D, MAXN, NCORES = 32768, 64, 128, 16, 8
PAIR = 1024          # rows per pair-group (2 partition halves x 512 cols)
COLS = PAIR // 2     # free-dim width of one pair tile

LAST_RESULTS = None  # BassKernelResults of the most recent run (for test.py)

_PROGRAM_CACHE = {}


def _blkdiag(a):
    out = np.zeros((128, 128), np.float32)
    out[:64, :64] = a
    out[64:, 64:] = a
    return out


def _build_program(S_pairs, V, B_pad):
    """Build the SPMD Bass program for the static schedule.

    S_pairs: steps to run for each pair-group (monotone non-increasing).
    V[t]:    number of valid rows at step t (same on every core).
    """
    npairs = len(S_pairs)
    nc = bacc.Bacc(None, target_bir_lowering=False)

    # ---- I/O ----------------------------------------------------------
    zt_d = nc.dram_tensor("zt", [128, npairs * COLS], BF, kind="ExternalInput")
    w_names = ["w_r0", "w_u0", "w_hc0", "w_r", "w_u", "w_ic", "w_hc",
               "w_i", "w_h1"]
    w_d = {k: nc.dram_tensor(k, [128, 128], BF, kind="ExternalInput")
           for k in w_names}
    w2_d = nc.dram_tensor("w2", [128, 128], BF, kind="ExternalInput")
    b2t_d = nc.dram_tensor("b2t", [128, 512], F32, kind="ExternalInput")
    bias_names = ["b_r", "b_u", "b_ic", "b_hc", "b_1"]
    bias_d = {k: nc.dram_tensor(k, [128, 1], F32, kind="ExternalInput")
              for k in bias_names}
    out_d = nc.dram_tensor("out", [B_pad, MAXN, D], F32, kind="ExternalOutput")

    with tile.TileContext(nc) as tc:
        with (
            tc.tile_pool(name="const", bufs=1) as cpool,
            tc.tile_pool(name="state", bufs=1) as spool,
            tc.tile_pool(name="work", bufs=3) as wpool,
            tc.tile_pool(name="outsb", bufs=2) as opool,
            tc.tile_pool(name="psum", bufs=2, space="PSUM") as ppool,
        )\
        :
            # ---- constants ------------------------------------------
            zt = cpool.tile([128, npairs * COLS], BF, name="zt_sb")
            nc.sync.dma_start(out=zt[:], in_=zt_d[:])
            w = {}
            for k in w_names:
                w[k] = cpool.tile([128, 128], BF, name=f"{k}_sb")
                nc.sync.dma_start(out=w[k][:], in_=w_d[k][:])
            w2 = cpool.tile([128, 128], BF, name="w2_sb")
            nc.sync.dma_start(out=w2[:], in_=w2_d[:])
            b2t = cpool.tile([128, 512], F32, name="b2t_sb")
            nc.sync.dma_start(out=b2t[:], in_=b2t_d[:])
            bias = {}
            for k in bias_names:
                bias[k] = cpool.tile([128, 1], F32, name=f"{k}_sb")
                nc.sync.dma_start(out=bias[k][:], in_=bias_d[k][:])

            hstate = spool.tile([128, npairs * COLS], BF, name="hstate")

            # ---- two-slot software pipeline over pair sequences -----
            # slot A runs pairs [0, 3], slot B runs [1, 2]: with S monotone
            # decreasing this balances the two slots' total step counts.
            seqA = list(range(0, npairs, 2))
            seqB = list(range(1, npairs, 2))
            if npairs == 4:
                seqA, seqB = [0, 3], [1, 2]
            schedA = [(p, t) for p in seqA for t in range(S_pairs[p])]
            schedB = [(p, t) for p in seqB for t in range(S_pairs[p])]
            rounds = max(len(schedA), len(schedB))

            for k in range(rounds):
                items = []
                if k < len(schedA):
                    items.append(schedA[k])
                if k < len(schedB):
                    items.append(schedB[k])

                tiles = {}
                for (p, t) in items:
                    hs = hstate[:, p * COLS:(p + 1) * COLS]
                    hin = zt[:, p * COLS:(p + 1) * COLS] if t == 0 else hs
                    rp = ppool.tile([128, COLS], F32, tag="rh1")
                    up = ppool.tile([128, COLS], F32, tag="uout")
                    hcp = ppool.tile([128, COLS], F32, tag="hc")
                    icrh = ppool.tile([128, COLS], F32, tag="icrh")
                    nc.tensor.matmul(rp[:],
                                     w["w_r0" if t == 0 else "w_r"][:], hin)
                    nc.tensor.matmul(up[:],
                                     w["w_u0" if t == 0 else "w_u"][:], hin)
                    nc.tensor.matmul(hcp[:],
                                     w["w_hc0" if t == 0 else "w_hc"][:], hin)
                    tiles[p] = (hs, hin, rp, up, hcp, icrh)

                for (p, t) in items:
                    hs, hin, rp, up, hcp, icrh = tiles[p]
                    r_sb = wpool.tile([128, COLS], BF, tag="r")
                    u_sb = wpool.tile([128, COLS], BF, tag="u")
                    nc.scalar.activation(r_sb[:], rp[:], ACTF.Sigmoid,
                                         bias=bias["b_r"][:])
                    nc.scalar.activation(u_sb[:], up[:], ACTF.Sigmoid,
                                         bias=bias["b_u"][:])
                    tiles[p] += (r_sb, u_sb)

                for (p, t) in items:
                    hs, hin, rp, up, hcp, icrh, r_sb, u_sb = tiles[p]
                    # rh = (hc_pre + b_hc) * r  -> written straight to PSUM
                    nc.vector.scalar_tensor_tensor(
                        icrh[:], hcp[:], bias["b_hc"][:], r_sb[:],
                        ALU.add, ALU.mult)
                    # ic matmul accumulates on top of rh (skipped at t==0)
                    if t > 0:
                        nc.tensor.matmul(icrh[:], w["w_ic"][:], hin,
                                         start=False, stop=True,
                                         skip_group_check=True)

                for (p, t) in items:
                    hs, hin, rp, up, hcp, icrh, r_sb, u_sb = tiles[p]
                    c_sb = wpool.tile([128, COLS], BF, tag="c")
                    nc.scalar.activation(c_sb[:], icrh[:], ACTF.Tanh,
                                         bias=bias["b_ic"][:])
                    e_sb = wpool.tile([128, COLS], BF, tag="e")
                    f_sb = wpool.tile([128, COLS], BF, tag="f")
                    nc.vector.tensor_tensor(e_sb[:], hin, c_sb[:],
                                            ALU.subtract)
                    nc.gpsimd.tensor_tensor(f_sb[:], u_sb[:], e_sb[:],
                                            ALU.mult)
                    nc.vector.tensor_tensor(hs, c_sb[:], f_sb[:], ALU.add)

                # ---- MLP + output for the steps of this round --------
                for (p, t) in items:
                    hs = hstate[:, p * COLS:(p + 1) * COLS]
                    h1p = ppool.tile([128, COLS], F32, tag="rh1")
                    nc.tensor.matmul(h1p[:], w["w_h1"][:], hs)
                    h1 = wpool.tile([128, COLS], BF, tag="h1")
                    nc.scalar.activation(h1[:], h1p[:], ACTF.Relu,
                                         bias=bias["b_1"][:])
                    for half in range(2):
                        outp = ppool.tile([128, COLS], F32, tag="uout")
                        for ck in range(4):
                            nc.tensor.matmul(
                                outp[:, ck * 128:(ck + 1) * 128],
                                h1[half * 64:(half + 1) * 64,
                                   ck * 128:(ck + 1) * 128],
                                w2[half * 64:(half + 1) * 64, :])
                        osb = opool.tile([128, COLS], F32, tag="osb")
                        nc.vector.tensor_tensor(osb[:], outp[:], b2t[:],
                                                ALU.add)
                        base = p * PAIR + half * COLS
                        vr = min(max(V[t] - base, 0), COLS)
                        cf, pr = vr // 128, vr % 128
                        if cf > 0:
                            nc.sync.dma_start(
                                out=out_d[base:base + cf * 128, t, :]
                                .rearrange("(c p) d -> p c d", p=128),
                                in_=osb[:, 0:cf * 128]
                                .rearrange("p (c d) -> p c d", d=128))
                        if pr > 0:
                            nc.sync.dma_start(
                                out=out_d[base + cf * 128:base + vr, t, :],
                                in_=osb[0:pr, cf * 128:cf * 128 + 128])
    nc.compile()
    return nc


# ---------------------------------------------------------------------
# host-side numpy GRU+MLP for the remainder rows
def _host_rows(z_r, n_r, W_ih, W_hh, b_ih, b_hh, W1, b1, W2, b2):
    R = z_r.shape[0]
    out = np.zeros((R, MAXN, D), np.float32)
    if R == 0:
        return out

    def sig(v):
        return 1.0 / (1.0 + np.exp(-v))

    x = np.zeros_like(z_r)
    h = z_r.copy()
    for t in range(MAXN):
        gi = x @ W_ih.T + b_ih
        gh = h @ W_hh.T + b_hh
        r = sig(gi[:, :64] + gh[:, :64])
        u = sig(gi[:, 64:128] + gh[:, 64:128])
        c = np.tanh(gi[:, 128:] + r * gh[:, 128:])
        h = (1.0 - u) * c + u * h
        x = h
        out[:, t, :] = np.maximum(h @ W1 + b1, 0.0) @ W2 + b2
    out *= (np.arange(MAXN)[None, :, None] < n_r[:, None, None])
    return out


def kernel(z, n, W_ih, W_hh, b_ih, b_hh, W1, b1, W2, b2):
    global LAST_RESULTS
    z = np.asarray(z, np.float32)
    n = np.asarray(n, np.int32)
    W_ih = np.asarray(W_ih, np.float32)
    W_hh = np.asarray(W_hh, np.float32)
    b_ih = np.asarray(b_ih, np.float32)
    b_hh = np.asarray(b_hh, np.float32)
    W1 = np.asarray(W1, np.float32)
    b1 = np.asarray(b1, np.float32)
    W2 = np.asarray(W2, np.float32)
    b2 = np.asarray(b2, np.float32)

    # ---- partition rows across cores with identical n-multisets ------
    core_rows = [[] for _ in range(NCORES)]
    leftover = []
    n_core_vals = []
    for v in range(MAXN - 1, 0, -1):
        idx = np.where(n == v)[0]
        k = len(idx) // NCORES
        for c in range(NCORES):
            core_rows[c].append(idx[c * k:(c + 1) * k])
        leftover.append(idx[NCORES * k:])
        n_core_vals.append(np.full(k, v, np.int32))
    core_rows = [np.concatenate(cr) for cr in core_rows]
    leftover = np.concatenate(leftover)
    n_core = np.concatenate(n_core_vals) if n_core_vals else np.zeros(0, np.int32)

    B_real = len(core_rows[0])
    B_pad = max(((B_real + PAIR - 1) // PAIR) * PAIR, PAIR)
    n_sched = np.zeros(B_pad, np.int32)
    n_sched[:B_real] = n_core
    npairs = B_pad // PAIR
    S_pairs = tuple(int(n_sched[p * PAIR]) for p in range(npairs))
    V = tuple(int((n_sched > t).sum()) for t in range(MAXN))

    # ---- weights / schedule -> device constants ----------------------
    Wr = (W_ih[0:64] + W_hh[0:64]).T
    Wu = (W_ih[64:128] + W_hh[64:128]).T
    Wic = W_ih[128:192].T
    Whc = W_hh[128:192].T
    consts = {
        "w_r0": _blkdiag(W_hh[0:64].T),
        "w_u0": _blkdiag(W_hh[64:128].T),
        "w_hc0": _blkdiag(W_hh[128:192].T),
        "w_r": _blkdiag(Wr),
        "w_u": _blkdiag(Wu),
        "w_ic": _blkdiag(Wic),
        "w_hc": _blkdiag(Whc),
        "w_i": np.eye(128, dtype=np.float32),
        "w_h1": _blkdiag(W1),
    }
    consts = {k: v.astype(BF16) for k, v in consts.items()}
    consts["w2"] = np.vstack([W2, W2]).astype(BF16)
    consts["b2t"] = np.tile(np.tile(b2, 4)[None, :], (128, 1)).astype(np.float32)
    consts["b_r"] = np.tile(b_ih[0:64] + b_hh[0:64], 2)[:, None].astype(np.float32)
    consts["b_u"] = np.tile(b_ih[64:128] + b_hh[64:128], 2)[:, None].astype(np.float32)
    consts["b_ic"] = np.tile(b_ih[128:192], 2)[:, None].astype(np.float32)
    consts["b_hc"] = np.tile(b_hh[128:192], 2)[:, None].astype(np.float32)
    consts["b_1"] = np.tile(b1, 2)[:, None].astype(np.float32)

    key = (S_pairs, V, B_pad)
    if key not in _PROGRAM_CACHE:
        _PROGRAM_CACHE.clear()
        _PROGRAM_CACHE[key] = _build_program(S_pairs, V, B_pad)
    nc = _PROGRAM_CACHE[key]

    in_maps = []
    for c in range(NCORES):
        zmat = np.zeros((B_pad, H), np.float32)
        zmat[:B_real] = z[core_rows[c]]
        zr = zmat.reshape(npairs, 2, COLS, H)
        ztc = zr.transpose(1, 3, 0, 2).reshape(128, npairs * COLS)
        m = dict(consts)
        m["zt"] = np.ascontiguousarray(ztc).astype(BF16)
        in_maps.append(m)

    res = run_bass_kernel_spmd(nc, in_maps, list(range(NCORES)))
    LAST_RESULTS = res

    # ---- gather ------------------------------------------------------
    x_out = np.zeros((B, MAXN, D), np.float32)
    for c in range(NCORES):
        x_out[core_rows[c]] = res.results[c]["out"][:B_real]
    if len(leftover):
        x_out[leftover] = _host_rows(z[leftover], n[leftover], W_ih, W_hh,
                                     b_ih, b_hh, W1, b1, W2, b2)
    mask = np.arange(MAXN)[None, :] < n[:, None]
    return x_out, mask


# revision 22
# speedup vs baseline: 1.6627x; 1.0307x over previous
"""Trainium2 Bass kernel for nn_Decoder (ragged GRU + MLP decoder).

Strategy
--------
Data-parallel over 8 NeuronCores.  Rows are bucketed by n (sequence
length) and distributed so that every core receives an IDENTICAL
multiset of n-values -> one static SPMD program works for all cores.
Rows with n == 0 never touch the device (output is zero).  A remainder
of < 120 rows (to make the per-value counts divisible by 8) is computed
on the host in numpy.

Per core, rows are sorted by n descending and grouped into "pairs" of
1024 rows.  The GRU state lives transposed in SBUF as [128, 512] bf16
tiles: partitions 0-63 hold h for rows [base, base+512), partitions
64-127 hold h for rows [base+512, base+1024).  Gate matmuls use
block-diagonal [128,128] bf16 weights so each gate for BOTH halves is
produced in a single N=512 matmul with the gate value for every row on
a distinct partition -> all elementwise ops run with 128 busy lanes.

Per pair only max(n in pair) GRU steps run (ragged skip).  Valid output
cells are written via exact-extent DMAs; everything else relies on the
PJRT path zero-initializing ExternalOutput buffers.

GRU algebra (x==h after step 0, PyTorch gate convention):
    r = sigmoid(Wr h + br)        Wr = (W_ih + W_hh)[0:64],   br = (b_ih+b_hh)[0:64]
    u = sigmoid(Wu h + bu)        Wu = (W_ih + W_hh)[64:128], bu = (b_ih+b_hh)[64:128]
    c = tanh(W_ic h + b_ic + r*(W_hc h + b_hc))
    h' = c + u*(h - c)
Step 0 uses x=0, h=z: W* = W_hh blocks only, same biases, ic = b_ic.
"""

import os
import sys

import numpy as np

sys.path.insert(0, "/opt/trn_rl_repo")

import ml_dtypes  # noqa: E402

BF16 = ml_dtypes.bfloat16

from concourse import bacc, bass, tile  # noqa: E402
from concourse.bass_utils import run_bass_kernel_spmd  # noqa: E402

mybir = bass.mybir
F32 = mybir.dt.float32
BF = mybir.dt.bfloat16
ALU = mybir.AluOpType
ACTF = mybir.ActivationFunctionType

B, H, D, MAXN, NCORES = 32768, 64, 128, 16, 8
PAIR = 1024          # rows per pair-group (2 partition halves x 512 cols)
COLS = PAIR // 2     # free-dim width of one pair tile

LAST_RESULTS = None  # BassKernelResults of the most recent run (for test.py)

_PROGRAM_CACHE = {}


def _blkdiag(a):
    out = np.zeros((128, 128), np.float32)
    out[:64, :64] = a
    out[64:, 64:] = a
    return out


def _build_program(S_pairs, V, B_pad):
    """Build the SPMD Bass program for the static schedule.

    S_pairs: steps to run for each pair-group (monotone non-increasing).
    V[t]:    number of valid rows at step t (same on every core).
    """
    npairs = len(S_pairs)
    nc = bacc.Bacc(None, target_bir_lowering=False)

    # ---- I/O ----------------------------------------------------------
    zt_d = nc.dram_tensor("zt", [128, npairs * COLS], BF, kind="ExternalInput")
    w_names = ["w_r0", "w_u0", "w_hc0", "w_r", "w_u", "w_ic", "w_hc",
               "w_i", "w_h1"]
    w_d = {k: nc.dram_tensor(k, [128, 128], BF, kind="ExternalInput")
           for k in w_names}
    w2_d = nc.dram_tensor("w2", [128, 128], BF, kind="ExternalInput")
    b2t_d = nc.dram_tensor("b2t", [128, 512], F32, kind="ExternalInput")
    bias_names = ["b_r", "b_u", "b_ic", "b_hc", "b_1"]
    bias_d = {k: nc.dram_tensor(k, [128, 1], F32, kind="ExternalInput")
              for k in bias_names}
    out_d = nc.dram_tensor("out", [B_pad, MAXN, D], F32, kind="ExternalOutput")

    with tile.TileContext(nc) as tc:
        with (
            tc.tile_pool(name="const", bufs=1) as cpool,
            tc.tile_pool(name="state", bufs=1) as spool,
            tc.tile_pool(name="work", bufs=3) as wpool,
            tc.tile_pool(name="outsb", bufs=2) as opool,
            tc.tile_pool(name="psum", bufs=2, space="PSUM") as ppool,
        )\
        :
            # ---- constants ------------------------------------------
            zt = cpool.tile([128, npairs * COLS], BF, name="zt_sb")
            nc.sync.dma_start(out=zt[:], in_=zt_d[:])
            w = {}
            for k in w_names:
                w[k] = cpool.tile([128, 128], BF, name=f"{k}_sb")
                nc.sync.dma_start(out=w[k][:], in_=w_d[k][:])
            w2 = cpool.tile([128, 128], BF, name="w2_sb")
            nc.sync.dma_start(out=w2[:], in_=w2_d[:])
            b2t = cpool.tile([128, 512], F32, name="b2t_sb")
            nc.sync.dma_start(out=b2t[:], in_=b2t_d[:])
            bias = {}
            for k in bias_names:
                bias[k] = cpool.tile([128, 1], F32, name=f"{k}_sb")
                nc.sync.dma_start(out=bias[k][:], in_=bias_d[k][:])

            hstate = spool.tile([128, npairs * COLS], BF, name="hstate")

            # ---- two-slot software pipeline over pair sequences -----
            # slot A runs pairs [0, 3], slot B runs [1, 2]: with S monotone
            # decreasing this balances the two slots' total step counts.
            seqA = list(range(0, npairs, 2))
            seqB = list(range(1, npairs, 2))
            if npairs == 4:
                seqA, seqB = [0, 3], [1, 2]
            schedA = [(p, t) for p in seqA for t in range(S_pairs[p])]
            schedB = [(p, t) for p in seqB for t in range(S_pairs[p])]
            rounds = max(len(schedA), len(schedB))

            for k in range(rounds):
                items = []
                if k < len(schedA):
                    items.append(schedA[k])
                if k < len(schedB):
                    items.append(schedB[k])

                # fw: op width = valid rows of half A (>= valid of half B)
                tiles = {}
                for (p, t) in items:
                    fw = min(max(V[t] - p * PAIR, 0), COLS)
                    hs = hstate[:, p * COLS:p * COLS + fw]
                    hin = zt[:, p * COLS:p * COLS + fw] if t == 0 else hs
                    rp = ppool.tile([128, COLS], F32, tag="rh1")
                    up = ppool.tile([128, COLS], F32, tag="uout")
                    hcp = ppool.tile([128, COLS], F32, tag="hc")
                    icrh = ppool.tile([128, COLS], F32, tag="icrh")
                    nc.tensor.matmul(rp[:, 0:fw],
                                     w["w_r0" if t == 0 else "w_r"][:], hin)
                    nc.tensor.matmul(up[:, 0:fw],
                                     w["w_u0" if t == 0 else "w_u"][:], hin)
                    nc.tensor.matmul(hcp[:, 0:fw],
                                     w["w_hc0" if t == 0 else "w_hc"][:], hin)
                    if t > 0:
                        nc.tensor.matmul(icrh[:, 0:fw], w["w_ic"][:], hin,
                                         start=True, stop=False,
                                         skip_group_check=True)
                    tiles[p] = (fw, hs, hin, rp, up, hcp, icrh)

                for (p, t) in items:
                    fw, hs, hin, rp, up, hcp, icrh = tiles[p]
                    r_sb = wpool.tile([128, COLS], BF, tag="r")
                    u_sb = wpool.tile([128, COLS], BF, tag="u")
                    nc.scalar.activation(r_sb[:, 0:fw], rp[:, 0:fw],
                                         ACTF.Sigmoid, bias=bias["b_r"][:])
                    nc.scalar.activation(u_sb[:, 0:fw], up[:, 0:fw],
                                         ACTF.Sigmoid, bias=bias["b_u"][:])
                    tiles[p] += (r_sb, u_sb)

                for (p, t) in items:
                    fw, hs, hin, rp, up, hcp, icrh, r_sb, u_sb = tiles[p]
                    # rh = (hc_pre + b_hc) * r
                    rh_sb = wpool.tile([128, COLS], BF, tag="rh")
                    nc.vector.scalar_tensor_tensor(
                        rh_sb[:, 0:fw], hcp[:, 0:fw], bias["b_hc"][:],
                        r_sb[:, 0:fw], ALU.add, ALU.mult)
                    # accumulate rh onto ic via identity weights (PE-only
                    # accumulation group keeps has_written semantics clean)
                    nc.tensor.matmul(icrh[:, 0:fw], w["w_i"][:],
                                     rh_sb[:, 0:fw],
                                     start=(t == 0), stop=True,
                                     skip_group_check=True)

                for (p, t) in items:
                    fw, hs, hin, rp, up, hcp, icrh, r_sb, u_sb = tiles[p]
                    c_sb = wpool.tile([128, COLS], BF, tag="c")
                    nc.scalar.activation(c_sb[:, 0:fw], icrh[:, 0:fw],
                                         ACTF.Tanh, bias=bias["b_ic"][:])
                    e_sb = wpool.tile([128, COLS], BF, tag="e")
                    f_sb = wpool.tile([128, COLS], BF, tag="f")
                    nc.vector.tensor_tensor(e_sb[:, 0:fw], hin,
                                            c_sb[:, 0:fw], ALU.subtract)
                    nc.gpsimd.tensor_tensor(f_sb[:, 0:fw], u_sb[:, 0:fw],
                                            e_sb[:, 0:fw], ALU.mult)
                    nc.vector.tensor_tensor(hs, c_sb[:, 0:fw],
                                            f_sb[:, 0:fw], ALU.add)

                # ---- MLP + output for the steps of this round --------
                for (p, t) in items:
                    fw = tiles[p][0]
                    hs = hstate[:, p * COLS:p * COLS + fw]
                    h1p = ppool.tile([128, COLS], F32, tag="rh1")
                    nc.tensor.matmul(h1p[:, 0:fw], w["w_h1"][:], hs)
                    h1 = wpool.tile([128, COLS], BF, tag="h1")
                    nc.scalar.activation(h1[:, 0:fw], h1p[:, 0:fw],
                                         ACTF.Relu, bias=bias["b_1"][:])
                    for half in range(2):
                        base = p * PAIR + half * COLS
                        vr = min(max(V[t] - base, 0), COLS)
                        if vr == 0:
                            continue
                        nck = (vr + 127) // 128
                        outp = ppool.tile([128, COLS], F32, tag="uout")
                        for ck in range(nck):
                            m = min(vr - ck * 128, 128)
                            nc.tensor.matmul(
                                outp[0:m, ck * 128:(ck + 1) * 128],
                                h1[half * 64:(half + 1) * 64,
                                   ck * 128:ck * 128 + m],
                                w2[half * 64:(half + 1) * 64, :])
                        osb = opool.tile([128, COLS], F32, tag="osb")
                        nc.vector.tensor_tensor(osb[:, 0:nck * 128],
                                                outp[:, 0:nck * 128],
                                                b2t[:, 0:nck * 128], ALU.add)
                        cf, pr = vr // 128, vr % 128
                        if cf > 0:
                            nc.sync.dma_start(
                                out=out_d[base:base + cf * 128, t, :]
                                .rearrange("(c p) d -> p c d", p=128),
                                in_=osb[:, 0:cf * 128]
                                .rearrange("p (c d) -> p c d", d=128))
                        if pr > 0:
                            nc.sync.dma_start(
                                out=out_d[base + cf * 128:base + vr, t, :],
                                in_=osb[0:pr, cf * 128:cf * 128 + 128])
    nc.compile()
    return nc


# ---------------------------------------------------------------------
# host-side numpy GRU+MLP for the remainder rows
def _host_rows(z_r, n_r, W_ih, W_hh, b_ih, b_hh, W1, b1, W2, b2):
    R = z_r.shape[0]
    out = np.zeros((R, MAXN, D), np.float32)
    if R == 0:
        return out

    def sig(v):
        return 1.0 / (1.0 + np.exp(-v))

    x = np.zeros_like(z_r)
    h = z_r.copy()
    for t in range(MAXN):
        gi = x @ W_ih.T + b_ih
        gh = h @ W_hh.T + b_hh
        r = sig(gi[:, :64] + gh[:, :64])
        u = sig(gi[:, 64:128] + gh[:, 64:128])
        c = np.tanh(gi[:, 128:] + r * gh[:, 128:])
        h = (1.0 - u) * c + u * h
        x = h
        out[:, t, :] = np.maximum(h @ W1 + b1, 0.0) @ W2 + b2
    out *= (np.arange(MAXN)[None, :, None] < n_r[:, None, None])
    return out


def kernel(z, n, W_ih, W_hh, b_ih, b_hh, W1, b1, W2, b2):
    global LAST_RESULTS
    z = np.asarray(z, np.float32)
    n = np.asarray(n, np.int32)
    W_ih = np.asarray(W_ih, np.float32)
    W_hh = np.asarray(W_hh, np.float32)
    b_ih = np.asarray(b_ih, np.float32)
    b_hh = np.asarray(b_hh, np.float32)
    W1 = np.asarray(W1, np.float32)
    b1 = np.asarray(b1, np.float32)
    W2 = np.asarray(W2, np.float32)
    b2 = np.asarray(b2, np.float32)

    # ---- partition rows across cores with identical n-multisets ------
    core_rows = [[] for _ in range(NCORES)]
    leftover = []
    n_core_vals = []
    for v in range(MAXN - 1, 0, -1):
        idx = np.where(n == v)[0]
        k = len(idx) // NCORES
        for c in range(NCORES):
            core_rows[c].append(idx[c * k:(c + 1) * k])
        leftover.append(idx[NCORES * k:])
        n_core_vals.append(np.full(k, v, np.int32))
    core_rows = [np.concatenate(cr) for cr in core_rows]
    leftover = np.concatenate(leftover)
    n_core = np.concatenate(n_core_vals) if n_core_vals else np.zeros(0, np.int32)

    B_real = len(core_rows[0])
    B_pad = max(((B_real + PAIR - 1) // PAIR) * PAIR, PAIR)
    n_sched = np.zeros(B_pad, np.int32)
    n_sched[:B_real] = n_core
    npairs = B_pad // PAIR
    S_pairs = tuple(int(n_sched[p * PAIR]) for p in range(npairs))
    V = tuple(int((n_sched > t).sum()) for t in range(MAXN))

    # ---- weights / schedule -> device constants ----------------------
    Wr = (W_ih[0:64] + W_hh[0:64]).T
    Wu = (W_ih[64:128] + W_hh[64:128]).T
    Wic = W_ih[128:192].T
    Whc = W_hh[128:192].T
    consts = {
        "w_r0": _blkdiag(W_hh[0:64].T),
        "w_u0": _blkdiag(W_hh[64:128].T),
        "w_hc0": _blkdiag(W_hh[128:192].T),
        "w_r": _blkdiag(Wr),
        "w_u": _blkdiag(Wu),
        "w_ic": _blkdiag(Wic),
        "w_hc": _blkdiag(Whc),
        "w_i": np.eye(128, dtype=np.float32),
        "w_h1": _blkdiag(W1),
    }
    consts = {k: v.astype(BF16) for k, v in consts.items()}
    consts["w2"] = np.vstack([W2, W2]).astype(BF16)
    consts["b2t"] = np.tile(np.tile(b2, 4)[None, :], (128, 1)).astype(np.float32)
    consts["b_r"] = np.tile(b_ih[0:64] + b_hh[0:64], 2)[:, None].astype(np.float32)
    consts["b_u"] = np.tile(b_ih[64:128] + b_hh[64:128], 2)[:, None].astype(np.float32)
    consts["b_ic"] = np.tile(b_ih[128:192], 2)[:, None].astype(np.float32)
    consts["b_hc"] = np.tile(b_hh[128:192], 2)[:, None].astype(np.float32)
    consts["b_1"] = np.tile(b1, 2)[:, None].astype(np.float32)

    key = (S_pairs, V, B_pad)
    if key not in _PROGRAM_CACHE:
        _PROGRAM_CACHE.clear()
        _PROGRAM_CACHE[key] = _build_program(S_pairs, V, B_pad)
    nc = _PROGRAM_CACHE[key]

    in_maps = []
    for c in range(NCORES):
        zmat = np.zeros((B_pad, H), np.float32)
        zmat[:B_real] = z[core_rows[c]]
        zr = zmat.reshape(npairs, 2, COLS, H)
        ztc = zr.transpose(1, 3, 0, 2).reshape(128, npairs * COLS)
        m = dict(consts)
        m["zt"] = np.ascontiguousarray(ztc).astype(BF16)
        in_maps.append(m)

    res = run_bass_kernel_spmd(nc, in_maps, list(range(NCORES)))
    LAST_RESULTS = res

    # ---- gather ------------------------------------------------------
    x_out = np.zeros((B, MAXN, D), np.float32)
    for c in range(NCORES):
        x_out[core_rows[c]] = res.results[c]["out"][:B_real]
    if len(leftover):
        x_out[leftover] = _host_rows(z[leftover], n[leftover], W_ih, W_hh,
                                     b_ih, b_hh, W1, b1, W2, b2)
    mask = np.arange(MAXN)[None, :] < n[:, None]
    return x_out, mask


# revision 24
# speedup vs baseline: 1.7395x; 1.0462x over previous
"""Trainium2 Bass kernel for nn_Decoder (ragged GRU + MLP decoder).

Strategy
--------
Data-parallel over 8 NeuronCores.  Rows are bucketed by n (sequence
length) and distributed so that every core receives an IDENTICAL
multiset of n-values -> one static SPMD program works for all cores.
Rows with n == 0 never touch the device (output is zero).  A remainder
of < 120 rows (to make the per-value counts divisible by 8) is computed
on the host in numpy.

Per core, rows are sorted by n descending and grouped into "pairs" of
1024 rows.  The GRU state lives transposed in SBUF as [128, 512] bf16
tiles: partitions 0-63 hold h for rows [base, base+512), partitions
64-127 hold h for rows [base+512, base+1024).  Gate matmuls use
block-diagonal [128,128] bf16 weights so each gate for BOTH halves is
produced in a single N=512 matmul with the gate value for every row on
a distinct partition -> all elementwise ops run with 128 busy lanes.

Per pair only max(n in pair) GRU steps run (ragged skip).  Valid output
cells are written via exact-extent DMAs; everything else relies on the
PJRT path zero-initializing ExternalOutput buffers.

GRU algebra (x==h after step 0, PyTorch gate convention):
    r = sigmoid(Wr h + br)        Wr = (W_ih + W_hh)[0:64],   br = (b_ih+b_hh)[0:64]
    u = sigmoid(Wu h + bu)        Wu = (W_ih + W_hh)[64:128], bu = (b_ih+b_hh)[64:128]
    c = tanh(W_ic h + b_ic + r*(W_hc h + b_hc))
    h' = c + u*(h - c)
Step 0 uses x=0, h=z: W* = W_hh blocks only, same biases, ic = b_ic.
"""

import os
import sys

import numpy as np

sys.path.insert(0, "/opt/trn_rl_repo")

import ml_dtypes  # noqa: E402

BF16 = ml_dtypes.bfloat16

from concourse import bacc, bass, tile  # noqa: E402
from concourse.bass_utils import run_bass_kernel_spmd  # noqa: E402

mybir = bass.mybir
F32 = mybir.dt.float32
BF = mybir.dt.bfloat16
ALU = mybir.AluOpType
ACTF = mybir.ActivationFunctionType

B, H, D, MAXN, NCORES = 32768, 64, 128, 16, 8
PAIR = 1024          # rows per pair-group (2 partition halves x 512 cols)
COLS = PAIR // 2     # free-dim width of one pair tile

LAST_RESULTS = None  # BassKernelResults of the most recent run (for test.py)

_PROGRAM_CACHE = {}


def _blkdiag(a):
    out = np.zeros((128, 128), np.float32)
    out[:64, :64] = a
    out[64:, 64:] = a
    return out


def _build_program(S_pairs, V, B_pad):
    """Build the SPMD Bass program for the static schedule.

    S_pairs: steps to run for each pair-group (monotone non-increasing).
    V[t]:    number of valid rows at step t (same on every core).
    """
    npairs = len(S_pairs)
    nc = bacc.Bacc(None, target_bir_lowering=False)

    # ---- I/O ----------------------------------------------------------
    zt_d = nc.dram_tensor("zt", [128, npairs * COLS], BF, kind="ExternalInput")
    w_names = ["w_r0", "w_u0", "w_hc0", "w_r", "w_u", "w_ic", "w_hc",
               "w_i", "w_h1"]
    w_d = {k: nc.dram_tensor(k, [128, 128], BF, kind="ExternalInput")
           for k in w_names}
    w2_d = nc.dram_tensor("w2", [128, 128], BF, kind="ExternalInput")
    b2t_d = nc.dram_tensor("b2t", [128, 512], F32, kind="ExternalInput")
    bias_names = ["b_r", "b_u", "b_ic", "b_hc", "b_1"]
    bias_d = {k: nc.dram_tensor(k, [128, 1], F32, kind="ExternalInput")
              for k in bias_names}
    out_d = nc.dram_tensor("out", [B_pad, MAXN, D], F32, kind="ExternalOutput")

    with tile.TileContext(nc) as tc:
        with (
            tc.tile_pool(name="const", bufs=1) as cpool,
            tc.tile_pool(name="state", bufs=1) as spool,
            tc.tile_pool(name="work", bufs=3) as wpool,
            tc.tile_pool(name="outsb", bufs=2) as opool,
            tc.tile_pool(name="psum", bufs=2, space="PSUM") as ppool,
        )\
        :
            # ---- constants ------------------------------------------
            zt = cpool.tile([128, npairs * COLS], BF, name="zt_sb")
            nc.sync.dma_start(out=zt[:], in_=zt_d[:])
            w = {}
            for k in w_names:
                w[k] = cpool.tile([128, 128], BF, name=f"{k}_sb")
                nc.sync.dma_start(out=w[k][:], in_=w_d[k][:])
            w2 = cpool.tile([128, 128], BF, name="w2_sb")
            nc.sync.dma_start(out=w2[:], in_=w2_d[:])
            b2t = cpool.tile([128, 512], F32, name="b2t_sb")
            nc.sync.dma_start(out=b2t[:], in_=b2t_d[:])
            bias = {}
            for k in bias_names:
                bias[k] = cpool.tile([128, 1], F32, name=f"{k}_sb")
                nc.sync.dma_start(out=bias[k][:], in_=bias_d[k][:])

            hstate = spool.tile([128, npairs * COLS], BF, name="hstate")

            # ---- two-slot software pipeline over pair sequences -----
            # slot A runs pairs [0, 3], slot B runs [1, 2]: with S monotone
            # decreasing this balances the two slots' total step counts.
            seqA = list(range(0, npairs, 2))
            seqB = list(range(1, npairs, 2))
            if npairs == 4:
                seqA, seqB = [0, 3], [1, 2]
            schedA = [(p, t) for p in seqA for t in range(S_pairs[p])]
            schedB = [(p, t) for p in seqB for t in range(S_pairs[p])]
            rounds = max(len(schedA), len(schedB))

            for k in range(rounds):
                items = []
                if k < len(schedA):
                    items.append(schedA[k])
                if k < len(schedB):
                    items.append(schedB[k])

                # fw: op width = valid rows of half A (>= valid of half B)
                tiles = {}
                for (p, t) in items:
                    fw = min(max(V[t] - p * PAIR, 0), COLS)
                    hs = hstate[:, p * COLS:p * COLS + fw]
                    hin = zt[:, p * COLS:p * COLS + fw] if t == 0 else hs
                    rp = ppool.tile([128, COLS], F32, tag="rh1")
                    up = ppool.tile([128, COLS], F32, tag="uout")
                    hcp = ppool.tile([128, COLS], F32, tag="hc")
                    icrh = ppool.tile([128, COLS], F32, tag="icrh")
                    nc.tensor.matmul(rp[:, 0:fw],
                                     w["w_r0" if t == 0 else "w_r"][:], hin)
                    nc.tensor.matmul(up[:, 0:fw],
                                     w["w_u0" if t == 0 else "w_u"][:], hin)
                    nc.tensor.matmul(hcp[:, 0:fw],
                                     w["w_hc0" if t == 0 else "w_hc"][:], hin)
                    if t > 0:
                        nc.tensor.matmul(icrh[:, 0:fw], w["w_ic"][:], hin,
                                         start=True, stop=False,
                                         skip_group_check=True)
                    tiles[p] = (fw, hs, hin, rp, up, hcp, icrh)

                for (p, t) in items:
                    fw, hs, hin, rp, up, hcp, icrh = tiles[p]
                    r_sb = wpool.tile([128, COLS], BF, tag="r")
                    u_sb = wpool.tile([128, COLS], BF, tag="u")
                    nc.scalar.activation(r_sb[:, 0:fw], rp[:, 0:fw],
                                         ACTF.Sigmoid, bias=bias["b_r"][:])
                    nc.scalar.activation(u_sb[:, 0:fw], up[:, 0:fw],
                                         ACTF.Sigmoid, bias=bias["b_u"][:])
                    tiles[p] += (r_sb, u_sb)

                for (p, t) in items:
                    fw, hs, hin, rp, up, hcp, icrh, r_sb, u_sb = tiles[p]
                    # rh = (hc_pre + b_hc) * r
                    rh_sb = wpool.tile([128, COLS], BF, tag="rh")
                    nc.vector.scalar_tensor_tensor(
                        rh_sb[:, 0:fw], hcp[:, 0:fw], bias["b_hc"][:],
                        r_sb[:, 0:fw], ALU.add, ALU.mult)
                    # accumulate rh onto ic via identity weights (PE-only
                    # accumulation group keeps has_written semantics clean)
                    nc.tensor.matmul(icrh[:, 0:fw], w["w_i"][:],
                                     rh_sb[:, 0:fw],
                                     start=(t == 0), stop=True,
                                     skip_group_check=True)

                for (p, t) in items:
                    fw, hs, hin, rp, up, hcp, icrh, r_sb, u_sb = tiles[p]
                    c_sb = wpool.tile([128, COLS], BF, tag="c")
                    nc.scalar.activation(c_sb[:, 0:fw], icrh[:, 0:fw],
                                         ACTF.Tanh, bias=bias["b_ic"][:])
                    e_sb = wpool.tile([128, COLS], BF, tag="e")
                    f_sb = wpool.tile([128, COLS], BF, tag="f")
                    nc.vector.tensor_tensor(e_sb[:, 0:fw], hin,
                                            c_sb[:, 0:fw], ALU.subtract)
                    nc.vector.tensor_tensor(f_sb[:, 0:fw], u_sb[:, 0:fw],
                                            e_sb[:, 0:fw], ALU.mult)
                    nc.vector.tensor_tensor(hs, c_sb[:, 0:fw],
                                            f_sb[:, 0:fw], ALU.add)

                # ---- MLP + output for the steps of this round --------
                for (p, t) in items:
                    fw = tiles[p][0]
                    hs = hstate[:, p * COLS:p * COLS + fw]
                    h1p = ppool.tile([128, COLS], F32, tag="rh1")
                    nc.tensor.matmul(h1p[:, 0:fw], w["w_h1"][:], hs)
                    h1 = wpool.tile([128, COLS], BF, tag="h1")
                    nc.scalar.activation(h1[:, 0:fw], h1p[:, 0:fw],
                                         ACTF.Relu, bias=bias["b_1"][:])
                    for half in range(2):
                        base = p * PAIR + half * COLS
                        vr = min(max(V[t] - base, 0), COLS)
                        if vr == 0:
                            continue
                        nck = (vr + 127) // 128
                        outp = ppool.tile([128, COLS], F32, tag="uout")
                        for ck in range(nck):
                            m = min(vr - ck * 128, 128)
                            nc.tensor.matmul(
                                outp[0:m, ck * 128:(ck + 1) * 128],
                                h1[half * 64:(half + 1) * 64,
                                   ck * 128:ck * 128 + m],
                                w2[half * 64:(half + 1) * 64, :])
                        osb = opool.tile([128, COLS], F32, tag="osb")
                        nc.vector.tensor_tensor(osb[:, 0:nck * 128],
                                                outp[:, 0:nck * 128],
                                                b2t[:, 0:nck * 128], ALU.add)
                        cf, pr = vr // 128, vr % 128
                        if cf > 0:
                            nc.gpsimd.dma_start(
                                out=out_d[base:base + cf * 128, t, :]
                                .rearrange("(c p) d -> p c d", p=128),
                                in_=osb[:, 0:cf * 128]
                                .rearrange("p (c d) -> p c d", d=128))
                        if pr > 0:
                            nc.gpsimd.dma_start(
                                out=out_d[base + cf * 128:base + vr, t, :],
                                in_=osb[0:pr, cf * 128:cf * 128 + 128])
    nc.compile()
    return nc


# ---------------------------------------------------------------------
# host-side numpy GRU+MLP for the remainder rows
def _host_rows(z_r, n_r, W_ih, W_hh, b_ih, b_hh, W1, b1, W2, b2):
    R = z_r.shape[0]
    out = np.zeros((R, MAXN, D), np.float32)
    if R == 0:
        return out

    def sig(v):
        return 1.0 / (1.0 + np.exp(-v))

    x = np.zeros_like(z_r)
    h = z_r.copy()
    for t in range(MAXN):
        gi = x @ W_ih.T + b_ih
        gh = h @ W_hh.T + b_hh
        r = sig(gi[:, :64] + gh[:, :64])
        u = sig(gi[:, 64:128] + gh[:, 64:128])
        c = np.tanh(gi[:, 128:] + r * gh[:, 128:])
        h = (1.0 - u) * c + u * h
        x = h
        out[:, t, :] = np.maximum(h @ W1 + b1, 0.0) @ W2 + b2
    out *= (np.arange(MAXN)[None, :, None] < n_r[:, None, None])
    return out


def kernel(z, n, W_ih, W_hh, b_ih, b_hh, W1, b1, W2, b2):
    global LAST_RESULTS
    z = np.asarray(z, np.float32)
    n = np.asarray(n, np.int32)
    W_ih = np.asarray(W_ih, np.float32)
    W_hh = np.asarray(W_hh, np.float32)
    b_ih = np.asarray(b_ih, np.float32)
    b_hh = np.asarray(b_hh, np.float32)
    W1 = np.asarray(W1, np.float32)
    b1 = np.asarray(b1, np.float32)
    W2 = np.asarray(W2, np.float32)
    b2 = np.asarray(b2, np.float32)

    # ---- partition rows across cores with identical n-multisets ------
    core_rows = [[] for _ in range(NCORES)]
    leftover = []
    n_core_vals = []
    for v in range(MAXN - 1, 0, -1):
        idx = np.where(n == v)[0]
        k = len(idx) // NCORES
        for c in range(NCORES):
            core_rows[c].append(idx[c * k:(c + 1) * k])
        leftover.append(idx[NCORES * k:])
        n_core_vals.append(np.full(k, v, np.int32))
    core_rows = [np.concatenate(cr) for cr in core_rows]
    leftover = np.concatenate(leftover)
    n_core = np.concatenate(n_core_vals) if n_core_vals else np.zeros(0, np.int32)

    B_real = len(core_rows[0])
    B_pad = max(((B_real + PAIR - 1) // PAIR) * PAIR, PAIR)
    n_sched = np.zeros(B_pad, np.int32)
    n_sched[:B_real] = n_core
    npairs = B_pad // PAIR
    S_pairs = tuple(int(n_sched[p * PAIR]) for p in range(npairs))
    V = tuple(int((n_sched > t).sum()) for t in range(MAXN))

    # ---- weights / schedule -> device constants ----------------------
    Wr = (W_ih[0:64] + W_hh[0:64]).T
    Wu = (W_ih[64:128] + W_hh[64:128]).T
    Wic = W_ih[128:192].T
    Whc = W_hh[128:192].T
    consts = {
        "w_r0": _blkdiag(W_hh[0:64].T),
        "w_u0": _blkdiag(W_hh[64:128].T),
        "w_hc0": _blkdiag(W_hh[128:192].T),
        "w_r": _blkdiag(Wr),
        "w_u": _blkdiag(Wu),
        "w_ic": _blkdiag(Wic),
        "w_hc": _blkdiag(Whc),
        "w_i": np.eye(128, dtype=np.float32),
        "w_h1": _blkdiag(W1),
    }
    consts = {k: v.astype(BF16) for k, v in consts.items()}
    consts["w2"] = np.vstack([W2, W2]).astype(BF16)
    consts["b2t"] = np.tile(np.tile(b2, 4)[None, :], (128, 1)).astype(np.float32)
    consts["b_r"] = np.tile(b_ih[0:64] + b_hh[0:64], 2)[:, None].astype(np.float32)
    consts["b_u"] = np.tile(b_ih[64:128] + b_hh[64:128], 2)[:, None].astype(np.float32)
    consts["b_ic"] = np.tile(b_ih[128:192], 2)[:, None].astype(np.float32)
    consts["b_hc"] = np.tile(b_hh[128:192], 2)[:, None].astype(np.float32)
    consts["b_1"] = np.tile(b1, 2)[:, None].astype(np.float32)

    key = (S_pairs, V, B_pad)
    if key not in _PROGRAM_CACHE:
        _PROGRAM_CACHE.clear()
        _PROGRAM_CACHE[key] = _build_program(S_pairs, V, B_pad)
    nc = _PROGRAM_CACHE[key]

    in_maps = []
    for c in range(NCORES):
        zmat = np.zeros((B_pad, H), np.float32)
        zmat[:B_real] = z[core_rows[c]]
        zr = zmat.reshape(npairs, 2, COLS, H)
        ztc = zr.transpose(1, 3, 0, 2).reshape(128, npairs * COLS)
        m = dict(consts)
        m["zt"] = np.ascontiguousarray(ztc).astype(BF16)
        in_maps.append(m)

    res = run_bass_kernel_spmd(nc, in_maps, list(range(NCORES)))
    LAST_RESULTS = res

    # ---- gather ------------------------------------------------------
    x_out = np.zeros((B, MAXN, D), np.float32)
    for c in range(NCORES):
        x_out[core_rows[c]] = res.results[c]["out"][:B_real]
    if len(leftover):
        x_out[leftover] = _host_rows(z[leftover], n[leftover], W_ih, W_hh,
                                     b_ih, b_hh, W1, b1, W2, b2)
    mask = np.arange(MAXN)[None, :] < n[:, None]
    return x_out, mask


# revision 25
# speedup vs baseline: 1.7562x; 1.0096x over previous
"""Trainium2 Bass kernel for nn_Decoder (ragged GRU + MLP decoder).

Strategy
--------
Data-parallel over 8 NeuronCores.  Rows are bucketed by n (sequence
length) and distributed so that every core receives an IDENTICAL
multiset of n-values -> one static SPMD program works for all cores.
Rows with n == 0 never touch the device (output is zero).  A remainder
of < 120 rows (to make the per-value counts divisible by 8) is computed
on the host in numpy.

Per core, rows are sorted by n descending and grouped into "pairs" of
1024 rows.  The GRU state lives transposed in SBUF as [128, 512] bf16
tiles: partitions 0-63 hold h for rows [base, base+512), partitions
64-127 hold h for rows [base+512, base+1024).  Gate matmuls use
block-diagonal [128,128] bf16 weights so each gate for BOTH halves is
produced in a single N=512 matmul with the gate value for every row on
a distinct partition -> all elementwise ops run with 128 busy lanes.

Per pair only max(n in pair) GRU steps run (ragged skip).  Valid output
cells are written via exact-extent DMAs; everything else relies on the
PJRT path zero-initializing ExternalOutput buffers.

GRU algebra (x==h after step 0, PyTorch gate convention):
    r = sigmoid(Wr h + br)        Wr = (W_ih + W_hh)[0:64],   br = (b_ih+b_hh)[0:64]
    u = sigmoid(Wu h + bu)        Wu = (W_ih + W_hh)[64:128], bu = (b_ih+b_hh)[64:128]
    c = tanh(W_ic h + b_ic + r*(W_hc h + b_hc))
    h' = c + u*(h - c)
Step 0 uses x=0, h=z: W* = W_hh blocks only, same biases, ic = b_ic.
"""

import os
import sys

import numpy as np

sys.path.insert(0, "/opt/trn_rl_repo")

import ml_dtypes  # noqa: E402

BF16 = ml_dtypes.bfloat16

from concourse import bacc, bass, tile  # noqa: E402
from concourse.bass_utils import run_bass_kernel_spmd  # noqa: E402

mybir = bass.mybir
F32 = mybir.dt.float32
BF = mybir.dt.bfloat16
ALU = mybir.AluOpType
ACTF = mybir.ActivationFunctionType

B, H, D, MAXN, NCORES = 32768, 64, 128, 16, 8
PAIR = 1024          # rows per pair-group (2 partition halves x 512 cols)
COLS = PAIR // 2     # free-dim width of one pair tile

LAST_RESULTS = None  # BassKernelResults of the most recent run (for test.py)

_PROGRAM_CACHE = {}


def _blkdiag(a):
    out = np.zeros((128, 128), np.float32)
    out[:64, :64] = a
    out[64:, 64:] = a
    return out


def _build_program(S_pairs, V, B_pad):
    """Build the SPMD Bass program for the static schedule.

    S_pairs: steps to run for each pair-group (monotone non-increasing).
    V[t]:    number of valid rows at step t (same on every core).
    """
    npairs = len(S_pairs)
    nc = bacc.Bacc(None, target_bir_lowering=False)

    # ---- I/O ----------------------------------------------------------
    zt_d = nc.dram_tensor("zt", [128, npairs * COLS], BF, kind="ExternalInput")
    w_names = ["w_r0", "w_u0", "w_hc0", "w_r", "w_u", "w_ic", "w_hc",
               "w_i", "w_h1"]
    w_d = {k: nc.dram_tensor(k, [128, 128], BF, kind="ExternalInput")
           for k in w_names}
    w2_d = nc.dram_tensor("w2", [128, 128], BF, kind="ExternalInput")
    b2t_d = nc.dram_tensor("b2t", [128, 512], F32, kind="ExternalInput")
    bias_names = ["b_r", "b_u", "b_ic", "b_hc", "b_1"]
    bias_d = {k: nc.dram_tensor(k, [128, 1], F32, kind="ExternalInput")
              for k in bias_names}
    out_d = nc.dram_tensor("out", [B_pad, MAXN, D], F32, kind="ExternalOutput")

    with tile.TileContext(nc) as tc:
        with (
            tc.tile_pool(name="const", bufs=1) as cpool,
            tc.tile_pool(name="state", bufs=1) as spool,
            tc.tile_pool(name="work", bufs=3) as wpool,
            tc.tile_pool(name="outsb", bufs=2) as opool,
            tc.tile_pool(name="psum", bufs=2, space="PSUM") as ppool,
        )\
        :
            # ---- constants ------------------------------------------
            zt = cpool.tile([128, npairs * COLS], BF, name="zt_sb")
            nc.sync.dma_start(out=zt[:], in_=zt_d[:])
            w = {}
            for k in w_names:
                w[k] = cpool.tile([128, 128], BF, name=f"{k}_sb")
                nc.sync.dma_start(out=w[k][:], in_=w_d[k][:])
            w2 = cpool.tile([128, 128], BF, name="w2_sb")
            nc.sync.dma_start(out=w2[:], in_=w2_d[:])
            b2t = cpool.tile([128, 512], F32, name="b2t_sb")
            nc.sync.dma_start(out=b2t[:], in_=b2t_d[:])
            bias = {}
            for k in bias_names:
                bias[k] = cpool.tile([128, 1], F32, name=f"{k}_sb")
                nc.sync.dma_start(out=bias[k][:], in_=bias_d[k][:])

            hstate = spool.tile([128, npairs * COLS], BF, name="hstate")

            # ---- two-slot software pipeline over pair sequences -----
            # slot A runs pairs [0, 3], slot B runs [1, 2]: with S monotone
            # decreasing this balances the two slots' total step counts.
            if npairs == 4:
                seqs = [[0], [1, 3], [2]]
            else:
                seqs = [[p] for p in range(npairs)]
            scheds = [[(p, t) for p in sq for t in range(S_pairs[p])]
                      for sq in seqs]
            rounds = max(len(sc) for sc in scheds)

            for k in range(rounds):
                items = [sc[k] for sc in scheds if k < len(sc)]

                # fw: op width = valid rows of half A (>= valid of half B)
                tiles = {}
                for (p, t) in items:
                    fw = min(max(V[t] - p * PAIR, 0), COLS)
                    hs = hstate[:, p * COLS:p * COLS + fw]
                    hin = zt[:, p * COLS:p * COLS + fw] if t == 0 else hs
                    rp = ppool.tile([128, COLS], F32, tag="rh1")
                    up = ppool.tile([128, COLS], F32, tag="uout")
                    hcp = ppool.tile([128, COLS], F32, tag="hc")
                    icrh = ppool.tile([128, COLS], F32, tag="icrh")
                    nc.tensor.matmul(rp[:, 0:fw],
                                     w["w_r0" if t == 0 else "w_r"][:], hin)
                    nc.tensor.matmul(up[:, 0:fw],
                                     w["w_u0" if t == 0 else "w_u"][:], hin)
                    nc.tensor.matmul(hcp[:, 0:fw],
                                     w["w_hc0" if t == 0 else "w_hc"][:], hin)
                    if t > 0:
                        nc.tensor.matmul(icrh[:, 0:fw], w["w_ic"][:], hin,
                                         start=True, stop=False,
                                         skip_group_check=True)
                    tiles[p] = (fw, hs, hin, rp, up, hcp, icrh)

                for (p, t) in items:
                    fw, hs, hin, rp, up, hcp, icrh = tiles[p]
                    r_sb = wpool.tile([128, COLS], BF, tag="r")
                    u_sb = wpool.tile([128, COLS], BF, tag="u")
                    nc.scalar.activation(r_sb[:, 0:fw], rp[:, 0:fw],
                                         ACTF.Sigmoid, bias=bias["b_r"][:])
                    nc.scalar.activation(u_sb[:, 0:fw], up[:, 0:fw],
                                         ACTF.Sigmoid, bias=bias["b_u"][:])
                    tiles[p] += (r_sb, u_sb)

                for (p, t) in items:
                    fw, hs, hin, rp, up, hcp, icrh, r_sb, u_sb = tiles[p]
                    # rh = (hc_pre + b_hc) * r
                    rh_sb = wpool.tile([128, COLS], BF, tag="rh")
                    nc.vector.scalar_tensor_tensor(
                        rh_sb[:, 0:fw], hcp[:, 0:fw], bias["b_hc"][:],
                        r_sb[:, 0:fw], ALU.add, ALU.mult)
                    # accumulate rh onto ic via identity weights (PE-only
                    # accumulation group keeps has_written semantics clean)
                    nc.tensor.matmul(icrh[:, 0:fw], w["w_i"][:],
                                     rh_sb[:, 0:fw],
                                     start=(t == 0), stop=True,
                                     skip_group_check=True)

                for (p, t) in items:
                    fw, hs, hin, rp, up, hcp, icrh, r_sb, u_sb = tiles[p]
                    c_sb = wpool.tile([128, COLS], BF, tag="c")
                    nc.scalar.activation(c_sb[:, 0:fw], icrh[:, 0:fw],
                                         ACTF.Tanh, bias=bias["b_ic"][:])
                    e_sb = wpool.tile([128, COLS], BF, tag="e")
                    f_sb = wpool.tile([128, COLS], BF, tag="f")
                    nc.vector.tensor_tensor(e_sb[:, 0:fw], hin,
                                            c_sb[:, 0:fw], ALU.subtract)
                    nc.vector.tensor_tensor(f_sb[:, 0:fw], u_sb[:, 0:fw],
                                            e_sb[:, 0:fw], ALU.mult)
                    nc.vector.tensor_tensor(hs, c_sb[:, 0:fw],
                                            f_sb[:, 0:fw], ALU.add)

                # ---- MLP + output for the steps of this round --------
                for (p, t) in items:
                    fw = tiles[p][0]
                    hs = hstate[:, p * COLS:p * COLS + fw]
                    h1p = ppool.tile([128, COLS], F32, tag="rh1")
                    nc.tensor.matmul(h1p[:, 0:fw], w["w_h1"][:], hs)
                    h1 = wpool.tile([128, COLS], BF, tag="h1")
                    nc.scalar.activation(h1[:, 0:fw], h1p[:, 0:fw],
                                         ACTF.Relu, bias=bias["b_1"][:])
                    for half in range(2):
                        base = p * PAIR + half * COLS
                        vr = min(max(V[t] - base, 0), COLS)
                        if vr == 0:
                            continue
                        nck = (vr + 127) // 128
                        outp = ppool.tile([128, COLS], F32, tag="uout")
                        for ck in range(nck):
                            m = min(vr - ck * 128, 128)
                            nc.tensor.matmul(
                                outp[0:m, ck * 128:(ck + 1) * 128],
                                h1[half * 64:(half + 1) * 64,
                                   ck * 128:ck * 128 + m],
                                w2[half * 64:(half + 1) * 64, :])
                        osb = opool.tile([128, COLS], F32, tag="osb")
                        nc.vector.tensor_tensor(osb[:, 0:nck * 128],
                                                outp[:, 0:nck * 128],
                                                b2t[:, 0:nck * 128], ALU.add)
                        cf, pr = vr // 128, vr % 128
                        if cf > 0:
                            nc.gpsimd.dma_start(
                                out=out_d[base:base + cf * 128, t, :]
                                .rearrange("(c p) d -> p c d", p=128),
                                in_=osb[:, 0:cf * 128]
                                .rearrange("p (c d) -> p c d", d=128))
                        if pr > 0:
                            nc.gpsimd.dma_start(
                                out=out_d[base + cf * 128:base + vr, t, :],
                                in_=osb[0:pr, cf * 128:cf * 128 + 128])
    nc.compile()
    return nc


# ---------------------------------------------------------------------
# host-side numpy GRU+MLP for the remainder rows
def _host_rows(z_r, n_r, W_ih, W_hh, b_ih, b_hh, W1, b1, W2, b2):
    R = z_r.shape[0]
    out = np.zeros((R, MAXN, D), np.float32)
    if R == 0:
        return out

    def sig(v):
        return 1.0 / (1.0 + np.exp(-v))

    x = np.zeros_like(z_r)
    h = z_r.copy()
    for t in range(MAXN):
        gi = x @ W_ih.T + b_ih
        gh = h @ W_hh.T + b_hh
        r = sig(gi[:, :64] + gh[:, :64])
        u = sig(gi[:, 64:128] + gh[:, 64:128])
        c = np.tanh(gi[:, 128:] + r * gh[:, 128:])
        h = (1.0 - u) * c + u * h
        x = h
        out[:, t, :] = np.maximum(h @ W1 + b1, 0.0) @ W2 + b2
    out *= (np.arange(MAXN)[None, :, None] < n_r[:, None, None])
    return out


def kernel(z, n, W_ih, W_hh, b_ih, b_hh, W1, b1, W2, b2):
    global LAST_RESULTS
    z = np.asarray(z, np.float32)
    n = np.asarray(n, np.int32)
    W_ih = np.asarray(W_ih, np.float32)
    W_hh = np.asarray(W_hh, np.float32)
    b_ih = np.asarray(b_ih, np.float32)
    b_hh = np.asarray(b_hh, np.float32)
    W1 = np.asarray(W1, np.float32)
    b1 = np.asarray(b1, np.float32)
    W2 = np.asarray(W2, np.float32)
    b2 = np.asarray(b2, np.float32)

    # ---- partition rows across cores with identical n-multisets ------
    core_rows = [[] for _ in range(NCORES)]
    leftover = []
    n_core_vals = []
    for v in range(MAXN - 1, 0, -1):
        idx = np.where(n == v)[0]
        k = len(idx) // NCORES
        for c in range(NCORES):
            core_rows[c].append(idx[c * k:(c + 1) * k])
        leftover.append(idx[NCORES * k:])
        n_core_vals.append(np.full(k, v, np.int32))
    core_rows = [np.concatenate(cr) for cr in core_rows]
    leftover = np.concatenate(leftover)
    n_core = np.concatenate(n_core_vals) if n_core_vals else np.zeros(0, np.int32)

    B_real = len(core_rows[0])
    B_pad = max(((B_real + PAIR - 1) // PAIR) * PAIR, PAIR)
    n_sched = np.zeros(B_pad, np.int32)
    n_sched[:B_real] = n_core
    npairs = B_pad // PAIR
    S_pairs = tuple(int(n_sched[p * PAIR]) for p in range(npairs))
    V = tuple(int((n_sched > t).sum()) for t in range(MAXN))

    # ---- weights / schedule -> device constants ----------------------
    Wr = (W_ih[0:64] + W_hh[0:64]).T
    Wu = (W_ih[64:128] + W_hh[64:128]).T
    Wic = W_ih[128:192].T
    Whc = W_hh[128:192].T
    consts = {
        "w_r0": _blkdiag(W_hh[0:64].T),
        "w_u0": _blkdiag(W_hh[64:128].T),
        "w_hc0": _blkdiag(W_hh[128:192].T),
        "w_r": _blkdiag(Wr),
        "w_u": _blkdiag(Wu),
        "w_ic": _blkdiag(Wic),
        "w_hc": _blkdiag(Whc),
        "w_i": np.eye(128, dtype=np.float32),
        "w_h1": _blkdiag(W1),
    }
    consts = {k: v.astype(BF16) for k, v in consts.items()}
    consts["w2"] = np.vstack([W2, W2]).astype(BF16)
    consts["b2t"] = np.tile(np.tile(b2, 4)[None, :], (128, 1)).astype(np.float32)
    consts["b_r"] = np.tile(b_ih[0:64] + b_hh[0:64], 2)[:, None].astype(np.float32)
    consts["b_u"] = np.tile(b_ih[64:128] + b_hh[64:128], 2)[:, None].astype(np.float32)
    consts["b_ic"] = np.tile(b_ih[128:192], 2)[:, None].astype(np.float32)
    consts["b_hc"] = np.tile(b_hh[128:192], 2)[:, None].astype(np.float32)
    consts["b_1"] = np.tile(b1, 2)[:, None].astype(np.float32)

    key = (S_pairs, V, B_pad)
    if key not in _PROGRAM_CACHE:
        _PROGRAM_CACHE.clear()
        _PROGRAM_CACHE[key] = _build_program(S_pairs, V, B_pad)
    nc = _PROGRAM_CACHE[key]

    in_maps = []
    for c in range(NCORES):
        zmat = np.zeros((B_pad, H), np.float32)
        zmat[:B_real] = z[core_rows[c]]
        zr = zmat.reshape(npairs, 2, COLS, H)
        ztc = zr.transpose(1, 3, 0, 2).reshape(128, npairs * COLS)
        m = dict(consts)
        m["zt"] = np.ascontiguousarray(ztc).astype(BF16)
        in_maps.append(m)

    res = run_bass_kernel_spmd(nc, in_maps, list(range(NCORES)))
    LAST_RESULTS = res

    # ---- gather ------------------------------------------------------
    x_out = np.zeros((B, MAXN, D), np.float32)
    for c in range(NCORES):
        x_out[core_rows[c]] = res.results[c]["out"][:B_real]
    if len(leftover):
        x_out[leftover] = _host_rows(z[leftover], n[leftover], W_ih, W_hh,
                                     b_ih, b_hh, W1, b1, W2, b2)
    mask = np.arange(MAXN)[None, :] < n[:, None]
    return x_out, mask


# revision 29
# speedup vs baseline: 1.8523x; 1.0547x over previous
"""Trainium2 Bass kernel for nn_Decoder (ragged GRU + MLP decoder).

Strategy
--------
Data-parallel over 8 NeuronCores.  Rows are bucketed by n (sequence
length) and distributed so that every core receives an IDENTICAL
multiset of n-values -> one static SPMD program works for all cores.
Rows with n == 0 never touch the device (output is zero).  A remainder
of < 120 rows (to make the per-value counts divisible by 8) is computed
on the host in numpy.

Per core, rows are sorted by n descending and grouped into "pairs" of
1024 rows.  The GRU state lives transposed in SBUF as [128, 512] bf16
tiles: partitions 0-63 hold h for rows [base, base+512), partitions
64-127 hold h for rows [base+512, base+1024).  Gate matmuls use
block-diagonal [128,128] bf16 weights so each gate for BOTH halves is
produced in a single N=512 matmul with the gate value for every row on
a distinct partition -> all elementwise ops run with 128 busy lanes.

Per pair only max(n in pair) GRU steps run (ragged skip).  Valid output
cells are written via exact-extent DMAs; everything else relies on the
PJRT path zero-initializing ExternalOutput buffers.

GRU algebra (x==h after step 0, PyTorch gate convention):
    r = sigmoid(Wr h + br)        Wr = (W_ih + W_hh)[0:64],   br = (b_ih+b_hh)[0:64]
    u = sigmoid(Wu h + bu)        Wu = (W_ih + W_hh)[64:128], bu = (b_ih+b_hh)[64:128]
    c = tanh(W_ic h + b_ic + r*(W_hc h + b_hc))
    h' = c + u*(h - c)
Step 0 uses x=0, h=z: W* = W_hh blocks only, same biases, ic = b_ic.
"""

import os
import sys

import numpy as np

sys.path.insert(0, "/opt/trn_rl_repo")

import ml_dtypes  # noqa: E402

BF16 = ml_dtypes.bfloat16

from concourse import bacc, bass, tile  # noqa: E402
from concourse.bass_utils import run_bass_kernel_spmd  # noqa: E402

mybir = bass.mybir
F32 = mybir.dt.float32
F16 = mybir.dt.float16
BF = mybir.dt.bfloat16
ALU = mybir.AluOpType
ACTF = mybir.ActivationFunctionType

B, H, D, MAXN, NCORES = 32768, 64, 128, 16, 8
PAIR = 1024          # rows per pair-group (2 partition halves x 512 cols)
COLS = PAIR // 2     # free-dim width of one pair tile

LAST_RESULTS = None  # BassKernelResults of the most recent run (for test.py)

_PROGRAM_CACHE = {}


def _blkdiag(a):
    out = np.zeros((128, 128), np.float32)
    out[:64, :64] = a
    out[64:, 64:] = a
    return out


def _build_program(S_pairs, V, B_pad):
    """Build the SPMD Bass program for the static schedule.

    S_pairs: steps to run for each pair-group (monotone non-increasing).
    V[t]:    number of valid rows at step t (same on every core).
    """
    npairs = len(S_pairs)
    nc = bacc.Bacc(None, target_bir_lowering=False)

    # ---- I/O ----------------------------------------------------------
    zt_d = nc.dram_tensor("zt", [128, npairs * COLS], BF, kind="ExternalInput")
    w_names = ["w_r0", "w_u0", "w_hc0", "w_r", "w_u", "w_ic", "w_hc",
               "w_i", "w_h1"]
    w_d = {k: nc.dram_tensor(k, [128, 128], BF, kind="ExternalInput")
           for k in w_names}
    w2_d = nc.dram_tensor("w2", [128, 128], BF, kind="ExternalInput")
    b2t_d = nc.dram_tensor("b2t", [128, 512], F32, kind="ExternalInput")
    bias_names = ["b_r", "b_u", "b_ic", "b_hc", "b_1"]
    bias_d = {k: nc.dram_tensor(k, [128, 1], F32, kind="ExternalInput")
              for k in bias_names}
    out_d = nc.dram_tensor("out", [B_pad, MAXN, D], F16, kind="ExternalOutput")

    with tile.TileContext(nc) as tc:
        with (
            tc.tile_pool(name="const", bufs=1) as cpool,
            tc.tile_pool(name="state", bufs=1) as spool,
            tc.tile_pool(name="work", bufs=3) as wpool,
            tc.tile_pool(name="outsb", bufs=2) as opool,
            tc.tile_pool(name="psum", bufs=2, space="PSUM") as ppool,
        )\
        :
            # ---- constants ------------------------------------------
            zt = cpool.tile([128, npairs * COLS], BF, name="zt_sb")
            nc.sync.dma_start(out=zt[:], in_=zt_d[:])
            w = {}
            for k in w_names:
                w[k] = cpool.tile([128, 128], BF, name=f"{k}_sb")
                nc.sync.dma_start(out=w[k][:], in_=w_d[k][:])
            w2 = cpool.tile([128, 128], BF, name="w2_sb")
            nc.sync.dma_start(out=w2[:], in_=w2_d[:])
            b2t = cpool.tile([128, 512], F32, name="b2t_sb")
            nc.sync.dma_start(out=b2t[:], in_=b2t_d[:])
            bias = {}
            for k in bias_names:
                bias[k] = cpool.tile([128, 1], F32, name=f"{k}_sb")
                nc.sync.dma_start(out=bias[k][:], in_=bias_d[k][:])

            hstate = spool.tile([128, npairs * COLS], BF, name="hstate")

            # ---- two-slot software pipeline over pair sequences -----
            # slot A runs pairs [0, 3], slot B runs [1, 2]: with S monotone
            # decreasing this balances the two slots' total step counts.
            if npairs == 4:
                seqs = [[0], [1, 3], [2]]
            else:
                seqs = [[p] for p in range(npairs)]
            scheds = [[(p, t) for p in sq for t in range(S_pairs[p])]
                      for sq in seqs]
            rounds = max(len(sc) for sc in scheds)

            for k in range(rounds):
                items = [sc[k] for sc in scheds if k < len(sc)]

                # fw: op width = valid rows of half A (>= valid of half B)
                tiles = {}
                for (p, t) in items:
                    fw = min(max(V[t] - p * PAIR, 0), COLS)
                    hs = hstate[:, p * COLS:p * COLS + fw]
                    hin = zt[:, p * COLS:p * COLS + fw] if t == 0 else hs
                    rp = ppool.tile([128, COLS], F32, tag="rh1")
                    up = ppool.tile([128, COLS], F32, tag="uout")
                    hcp = ppool.tile([128, COLS], F32, tag="hc")
                    icrh = ppool.tile([128, COLS], F32, tag="icrh")
                    nc.tensor.matmul(rp[:, 0:fw],
                                     w["w_r0" if t == 0 else "w_r"][:], hin)
                    nc.tensor.matmul(up[:, 0:fw],
                                     w["w_u0" if t == 0 else "w_u"][:], hin)
                    nc.tensor.matmul(hcp[:, 0:fw],
                                     w["w_hc0" if t == 0 else "w_hc"][:], hin)
                    if t > 0:
                        nc.tensor.matmul(icrh[:, 0:fw], w["w_ic"][:], hin,
                                         start=True, stop=False,
                                         skip_group_check=True)
                    tiles[p] = (fw, hs, hin, rp, up, hcp, icrh)

                for (p, t) in items:
                    fw, hs, hin, rp, up, hcp, icrh = tiles[p]
                    r_sb = wpool.tile([128, COLS], BF, tag="r")
                    u_sb = wpool.tile([128, COLS], BF, tag="u")
                    nc.scalar.activation(r_sb[:, 0:fw], rp[:, 0:fw],
                                         ACTF.Sigmoid, bias=bias["b_r"][:])
                    nc.scalar.activation(u_sb[:, 0:fw], up[:, 0:fw],
                                         ACTF.Sigmoid, bias=bias["b_u"][:])
                    tiles[p] += (r_sb, u_sb)

                for (p, t) in items:
                    fw, hs, hin, rp, up, hcp, icrh, r_sb, u_sb = tiles[p]
                    # rh = (hc_pre + b_hc) * r
                    rh_sb = wpool.tile([128, COLS], BF, tag="rh")
                    nc.vector.scalar_tensor_tensor(
                        rh_sb[:, 0:fw], hcp[:, 0:fw], bias["b_hc"][:],
                        r_sb[:, 0:fw], ALU.add, ALU.mult)
                    # accumulate rh onto ic via identity weights (PE-only
                    # accumulation group keeps has_written semantics clean)
                    nc.tensor.matmul(icrh[:, 0:fw], w["w_i"][:],
                                     rh_sb[:, 0:fw],
                                     start=(t == 0), stop=True,
                                     skip_group_check=True)

                for (p, t) in items:
                    fw, hs, hin, rp, up, hcp, icrh, r_sb, u_sb = tiles[p]
                    c_sb = wpool.tile([128, COLS], BF, tag="c")
                    nc.scalar.activation(c_sb[:, 0:fw], icrh[:, 0:fw],
                                         ACTF.Tanh, bias=bias["b_ic"][:])
                    e_sb = wpool.tile([128, COLS], BF, tag="e")
                    f_sb = wpool.tile([128, COLS], BF, tag="f")
                    nc.vector.tensor_tensor(e_sb[:, 0:fw], hin,
                                            c_sb[:, 0:fw], ALU.subtract)
                    nc.vector.tensor_tensor(f_sb[:, 0:fw], u_sb[:, 0:fw],
                                            e_sb[:, 0:fw], ALU.mult)
                    nc.vector.tensor_tensor(hs, c_sb[:, 0:fw],
                                            f_sb[:, 0:fw], ALU.add)

                # ---- MLP + output for the steps of this round --------
                for (p, t) in items:
                    fw = tiles[p][0]
                    hs = hstate[:, p * COLS:p * COLS + fw]
                    h1p = ppool.tile([128, COLS], F32, tag="rh1")
                    nc.tensor.matmul(h1p[:, 0:fw], w["w_h1"][:], hs)
                    h1 = wpool.tile([128, COLS], BF, tag="h1")
                    nc.scalar.activation(h1[:, 0:fw], h1p[:, 0:fw],
                                         ACTF.Relu, bias=bias["b_1"][:])
                    for half in range(2):
                        base = p * PAIR + half * COLS
                        vr = min(max(V[t] - base, 0), COLS)
                        if vr == 0:
                            continue
                        nck = (vr + 127) // 128
                        outp = ppool.tile([128, COLS], F32, tag="uout")
                        for ck in range(nck):
                            m = min(vr - ck * 128, 128)
                            nc.tensor.matmul(
                                outp[0:m, ck * 128:(ck + 1) * 128],
                                h1[half * 64:(half + 1) * 64,
                                   ck * 128:ck * 128 + m],
                                w2[half * 64:(half + 1) * 64, :])
                        osb = opool.tile([128, COLS], F16, tag="osb")
                        nc.vector.tensor_tensor(osb[:, 0:nck * 128],
                                                outp[:, 0:nck * 128],
                                                b2t[:, 0:nck * 128], ALU.add)
                        dmae = nc.gpsimd if (p + half) % 2 else nc.sync
                        cf, pr = vr // 128, vr % 128
                        if cf > 0:
                            dmae.dma_start(
                                out=out_d[base:base + cf * 128, t, :]
                                .rearrange("(c p) d -> p c d", p=128),
                                in_=osb[:, 0:cf * 128]
                                .rearrange("p (c d) -> p c d", d=128))
                        if pr > 0:
                            dmae.dma_start(
                                out=out_d[base + cf * 128:base + vr, t, :],
                                in_=osb[0:pr, cf * 128:cf * 128 + 128])
    nc.compile()
    return nc


# ---------------------------------------------------------------------
# host-side numpy GRU+MLP for the remainder rows
def _host_rows(z_r, n_r, W_ih, W_hh, b_ih, b_hh, W1, b1, W2, b2):
    R = z_r.shape[0]
    out = np.zeros((R, MAXN, D), np.float32)
    if R == 0:
        return out

    def sig(v):
        return 1.0 / (1.0 + np.exp(-v))

    x = np.zeros_like(z_r)
    h = z_r.copy()
    for t in range(MAXN):
        gi = x @ W_ih.T + b_ih
        gh = h @ W_hh.T + b_hh
        r = sig(gi[:, :64] + gh[:, :64])
        u = sig(gi[:, 64:128] + gh[:, 64:128])
        c = np.tanh(gi[:, 128:] + r * gh[:, 128:])
        h = (1.0 - u) * c + u * h
        x = h
        out[:, t, :] = np.maximum(h @ W1 + b1, 0.0) @ W2 + b2
    out *= (np.arange(MAXN)[None, :, None] < n_r[:, None, None])
    return out


def kernel(z, n, W_ih, W_hh, b_ih, b_hh, W1, b1, W2, b2):
    global LAST_RESULTS
    z = np.asarray(z, np.float32)
    n = np.asarray(n, np.int32)
    W_ih = np.asarray(W_ih, np.float32)
    W_hh = np.asarray(W_hh, np.float32)
    b_ih = np.asarray(b_ih, np.float32)
    b_hh = np.asarray(b_hh, np.float32)
    W1 = np.asarray(W1, np.float32)
    b1 = np.asarray(b1, np.float32)
    W2 = np.asarray(W2, np.float32)
    b2 = np.asarray(b2, np.float32)

    # ---- partition rows across cores with identical n-multisets ------
    core_rows = [[] for _ in range(NCORES)]
    leftover = []
    n_core_vals = []
    for v in range(MAXN - 1, 0, -1):
        idx = np.where(n == v)[0]
        k = len(idx) // NCORES
        for c in range(NCORES):
            core_rows[c].append(idx[c * k:(c + 1) * k])
        leftover.append(idx[NCORES * k:])
        n_core_vals.append(np.full(k, v, np.int32))
    core_rows = [np.concatenate(cr) for cr in core_rows]
    leftover = np.concatenate(leftover)
    n_core = np.concatenate(n_core_vals) if n_core_vals else np.zeros(0, np.int32)

    B_real = len(core_rows[0])
    B_pad = max(((B_real + PAIR - 1) // PAIR) * PAIR, PAIR)
    n_sched = np.zeros(B_pad, np.int32)
    n_sched[:B_real] = n_core
    npairs = B_pad // PAIR
    S_pairs = tuple(int(n_sched[p * PAIR]) for p in range(npairs))
    V = tuple(int((n_sched > t).sum()) for t in range(MAXN))

    # ---- weights / schedule -> device constants ----------------------
    Wr = (W_ih[0:64] + W_hh[0:64]).T
    Wu = (W_ih[64:128] + W_hh[64:128]).T
    Wic = W_ih[128:192].T
    Whc = W_hh[128:192].T
    consts = {
        "w_r0": _blkdiag(W_hh[0:64].T),
        "w_u0": _blkdiag(W_hh[64:128].T),
        "w_hc0": _blkdiag(W_hh[128:192].T),
        "w_r": _blkdiag(Wr),
        "w_u": _blkdiag(Wu),
        "w_ic": _blkdiag(Wic),
        "w_hc": _blkdiag(Whc),
        "w_i": np.eye(128, dtype=np.float32),
        "w_h1": _blkdiag(W1),
    }
    consts = {k: v.astype(BF16) for k, v in consts.items()}
    consts["w2"] = np.vstack([W2, W2]).astype(BF16)
    consts["b2t"] = np.tile(np.tile(b2, 4)[None, :], (128, 1)).astype(np.float32)
    consts["b_r"] = np.tile(b_ih[0:64] + b_hh[0:64], 2)[:, None].astype(np.float32)
    consts["b_u"] = np.tile(b_ih[64:128] + b_hh[64:128], 2)[:, None].astype(np.float32)
    consts["b_ic"] = np.tile(b_ih[128:192], 2)[:, None].astype(np.float32)
    consts["b_hc"] = np.tile(b_hh[128:192], 2)[:, None].astype(np.float32)
    consts["b_1"] = np.tile(b1, 2)[:, None].astype(np.float32)

    key = (S_pairs, V, B_pad)
    if key not in _PROGRAM_CACHE:
        _PROGRAM_CACHE.clear()
        _PROGRAM_CACHE[key] = _build_program(S_pairs, V, B_pad)
    nc = _PROGRAM_CACHE[key]

    in_maps = []
    for c in range(NCORES):
        zmat = np.zeros((B_pad, H), np.float32)
        zmat[:B_real] = z[core_rows[c]]
        zr = zmat.reshape(npairs, 2, COLS, H)
        ztc = zr.transpose(1, 3, 0, 2).reshape(128, npairs * COLS)
        m = dict(consts)
        m["zt"] = np.ascontiguousarray(ztc).astype(BF16)
        in_maps.append(m)

    res = run_bass_kernel_spmd(nc, in_maps, list(range(NCORES)))
    LAST_RESULTS = res

    # ---- gather ------------------------------------------------------
    x_out = np.zeros((B, MAXN, D), np.float32)
    for c in range(NCORES):
        x_out[core_rows[c]] = res.results[c]["out"][:B_real].astype(np.float32)
    if len(leftover):
        x_out[leftover] = _host_rows(z[leftover], n[leftover], W_ih, W_hh,
                                     b_ih, b_hh, W1, b1, W2, b2)
    mask = np.arange(MAXN)[None, :] < n[:, None]
    return x_out, mask


# revision 31
# speedup vs baseline: 2.1834x; 1.1787x over previous
"""Trainium2 Bass kernel for nn_Decoder (ragged GRU + MLP decoder).

Strategy
--------
Data-parallel over 8 NeuronCores.  Rows are bucketed by n (sequence
length) and distributed so that every core receives an IDENTICAL
multiset of n-values -> one static SPMD program works for all cores.
Rows with n == 0 never touch the device (output is zero).  A remainder
of < 120 rows (to make the per-value counts divisible by 8) is computed
on the host in numpy.

Per core, rows are sorted by n descending and grouped into "pairs" of
1024 rows.  The GRU state lives transposed in SBUF as [128, 512] bf16
tiles: partitions 0-63 hold h for rows [base, base+512), partitions
64-127 hold h for rows [base+512, base+1024).  Gate matmuls use
block-diagonal [128,128] bf16 weights so each gate for BOTH halves is
produced in a single N=512 matmul with the gate value for every row on
a distinct partition -> all elementwise ops run with 128 busy lanes.

Per pair only max(n in pair) GRU steps run (ragged skip).  Valid output
cells are written via exact-extent DMAs; everything else relies on the
PJRT path zero-initializing ExternalOutput buffers.

GRU algebra (x==h after step 0, PyTorch gate convention):
    r = sigmoid(Wr h + br)        Wr = (W_ih + W_hh)[0:64],   br = (b_ih+b_hh)[0:64]
    u = sigmoid(Wu h + bu)        Wu = (W_ih + W_hh)[64:128], bu = (b_ih+b_hh)[64:128]
    c = tanh(W_ic h + b_ic + r*(W_hc h + b_hc))
    h' = c + u*(h - c)
Step 0 uses x=0, h=z: W* = W_hh blocks only, same biases, ic = b_ic.
"""

import os
import sys

import numpy as np

sys.path.insert(0, "/opt/trn_rl_repo")

import ml_dtypes  # noqa: E402

BF16 = ml_dtypes.bfloat16

# bass_utils' trace path imports antenv.axon_hooks, which this image's
# antenv lacks; register a shim so BASS_TRACE=1 works (and can't crash).
try:
    import antenv.axon_hooks  # noqa: F401
except ImportError:
    import types

    _hook_holder = [None]
    _m = types.ModuleType("antenv.axon_hooks")
    _m.set_axon_ntff_profile_hook = lambda h: _hook_holder.__setitem__(0, h)
    _m.get_axon_ntff_profile_hook = lambda: _hook_holder[0]
    sys.modules["antenv.axon_hooks"] = _m
    try:
        import antenv

        antenv.axon_hooks = _m
        from trn_agent_boot.trn_boot import _ntff_profile_via_ctypes
        _m.set_axon_ntff_profile_hook(
            _ntff_profile_via_ctypes("/opt/axon/libaxon_pjrt.so"))
    except Exception:
        pass

from concourse import bacc, bass, tile  # noqa: E402
from concourse.bass_utils import run_bass_kernel_spmd  # noqa: E402

mybir = bass.mybir
F32 = mybir.dt.float32
F16 = mybir.dt.float16
BF = mybir.dt.bfloat16
ALU = mybir.AluOpType
ACTF = mybir.ActivationFunctionType

B, H, D, MAXN, NCORES = 32768, 64, 128, 16, 8
PAIR = 1024          # rows per pair-group (2 partition halves x 512 cols)
COLS = PAIR // 2     # free-dim width of one pair tile

LAST_RESULTS = None  # BassKernelResults of the most recent run (for test.py)

_PROGRAM_CACHE = {}


def _blkdiag(a):
    out = np.zeros((128, 128), np.float32)
    out[:64, :64] = a
    out[64:, 64:] = a
    return out


def _build_program(S_pairs, V, B_pad):
    """Build the SPMD Bass program for the static schedule.

    S_pairs: steps to run for each pair-group (monotone non-increasing).
    V[t]:    number of valid rows at step t (same on every core).
    """
    npairs = len(S_pairs)
    nc = bacc.Bacc(None, target_bir_lowering=False)

    # ---- I/O ----------------------------------------------------------
    zt_d = nc.dram_tensor("zt", [128, npairs * COLS], BF, kind="ExternalInput")
    w_names = ["w_r0", "w_u0", "w_hc0", "w_r", "w_u", "w_ic", "w_hc",
               "w_i", "w_h1"]
    w_d = {k: nc.dram_tensor(k, [128, 128], BF, kind="ExternalInput")
           for k in w_names}
    w2_d = nc.dram_tensor("w2", [128, 128], BF, kind="ExternalInput")
    b2t_d = nc.dram_tensor("b2t", [128, 512], F32, kind="ExternalInput")
    bias_names = ["b_r", "b_u", "b_ic", "b_hc", "b_1"]
    bias_d = {k: nc.dram_tensor(k, [128, 1], F32, kind="ExternalInput")
              for k in bias_names}
    out_d = nc.dram_tensor("out", [B_pad, MAXN, D], F16, kind="ExternalOutput")

    with tile.TileContext(nc) as tc:
        with (
            tc.tile_pool(name="const", bufs=1) as cpool,
            tc.tile_pool(name="state", bufs=1) as spool,
            tc.tile_pool(name="work", bufs=4) as wpool,
            tc.tile_pool(name="outsb", bufs=3) as opool,
            tc.tile_pool(name="psum", bufs=2, space="PSUM") as ppool,
        )\
        :
            # ---- constants ------------------------------------------
            zt = cpool.tile([128, npairs * COLS], BF, name="zt_sb")
            nc.sync.dma_start(out=zt[:], in_=zt_d[:])
            w = {}
            for k in w_names:
                w[k] = cpool.tile([128, 128], BF, name=f"{k}_sb")
                nc.sync.dma_start(out=w[k][:], in_=w_d[k][:])
            w2 = cpool.tile([128, 128], BF, name="w2_sb")
            nc.sync.dma_start(out=w2[:], in_=w2_d[:])
            b2t = cpool.tile([128, 512], F32, name="b2t_sb")
            nc.sync.dma_start(out=b2t[:], in_=b2t_d[:])
            bias = {}
            for k in bias_names:
                bias[k] = cpool.tile([128, 1], F32, name=f"{k}_sb")
                nc.sync.dma_start(out=bias[k][:], in_=bias_d[k][:])

            hstate = spool.tile([128, npairs * COLS], BF, name="hstate")

            # ---- two-slot software pipeline over pair sequences -----
            # slot A runs pairs [0, 3], slot B runs [1, 2]: with S monotone
            # decreasing this balances the two slots' total step counts.
            if npairs == 4:
                seqs = [[0], [1, 3], [2]]
            else:
                seqs = [[p] for p in range(npairs)]
            scheds = [[(p, t) for p in sq for t in range(S_pairs[p])]
                      for sq in seqs]
            rounds = max(len(sc) for sc in scheds)

            for k in range(rounds):
                items = [sc[k] for sc in scheds if k < len(sc)]

                # fw: op width = valid rows of half A (>= valid of half B)
                tiles = {}
                for (p, t) in items:
                    fw = min(max(V[t] - p * PAIR, 0), COLS)
                    hs = hstate[:, p * COLS:p * COLS + fw]
                    hin = zt[:, p * COLS:p * COLS + fw] if t == 0 else hs
                    rp = ppool.tile([128, COLS], F32, tag="rh1")
                    up = ppool.tile([128, COLS], F32, tag="uout")
                    hcp = ppool.tile([128, COLS], F32, tag="hc")
                    icrh = ppool.tile([128, COLS], F32, tag="icrh")
                    nc.tensor.matmul(rp[:, 0:fw],
                                     w["w_r0" if t == 0 else "w_r"][:], hin)
                    nc.tensor.matmul(up[:, 0:fw],
                                     w["w_u0" if t == 0 else "w_u"][:], hin)
                    nc.tensor.matmul(hcp[:, 0:fw],
                                     w["w_hc0" if t == 0 else "w_hc"][:], hin)
                    if t > 0:
                        nc.tensor.matmul(icrh[:, 0:fw], w["w_ic"][:], hin,
                                         start=True, stop=False,
                                         skip_group_check=True)
                    tiles[p] = (fw, hs, hin, rp, up, hcp, icrh)

                for (p, t) in items:
                    fw, hs, hin, rp, up, hcp, icrh = tiles[p]
                    r_sb = wpool.tile([128, COLS], BF, tag="r")
                    u_sb = wpool.tile([128, COLS], BF, tag="u")
                    nc.scalar.activation(r_sb[:, 0:fw], rp[:, 0:fw],
                                         ACTF.Sigmoid, bias=bias["b_r"][:])
                    nc.scalar.activation(u_sb[:, 0:fw], up[:, 0:fw],
                                         ACTF.Sigmoid, bias=bias["b_u"][:])
                    tiles[p] += (r_sb, u_sb)

                for (p, t) in items:
                    fw, hs, hin, rp, up, hcp, icrh, r_sb, u_sb = tiles[p]
                    # rh = (hc_pre + b_hc) * r
                    rh_sb = wpool.tile([128, COLS], BF, tag="rh")
                    nc.vector.scalar_tensor_tensor(
                        rh_sb[:, 0:fw], hcp[:, 0:fw], bias["b_hc"][:],
                        r_sb[:, 0:fw], ALU.add, ALU.mult)
                    # accumulate rh onto ic via identity weights (PE-only
                    # accumulation group keeps has_written semantics clean)
                    nc.tensor.matmul(icrh[:, 0:fw], w["w_i"][:],
                                     rh_sb[:, 0:fw],
                                     start=(t == 0), stop=True,
                                     skip_group_check=True)

                for (p, t) in items:
                    fw, hs, hin, rp, up, hcp, icrh, r_sb, u_sb = tiles[p]
                    c_sb = wpool.tile([128, COLS], BF, tag="c")
                    nc.scalar.activation(c_sb[:, 0:fw], icrh[:, 0:fw],
                                         ACTF.Tanh, bias=bias["b_ic"][:])
                    e_sb = wpool.tile([128, COLS], BF, tag="e")
                    f_sb = wpool.tile([128, COLS], BF, tag="f")
                    nc.vector.tensor_tensor(e_sb[:, 0:fw], hin,
                                            c_sb[:, 0:fw], ALU.subtract)
                    nc.vector.tensor_tensor(f_sb[:, 0:fw], u_sb[:, 0:fw],
                                            e_sb[:, 0:fw], ALU.mult)
                    nc.vector.tensor_tensor(hs, c_sb[:, 0:fw],
                                            f_sb[:, 0:fw], ALU.add)

                # ---- MLP + output for the steps of this round --------
                for (p, t) in items:
                    fw = tiles[p][0]
                    hs = hstate[:, p * COLS:p * COLS + fw]
                    h1p = ppool.tile([128, COLS], F32, tag="rh1")
                    nc.tensor.matmul(h1p[:, 0:fw], w["w_h1"][:], hs)
                    h1 = wpool.tile([128, COLS], BF, tag="h1")
                    nc.scalar.activation(h1[:, 0:fw], h1p[:, 0:fw],
                                         ACTF.Relu, bias=bias["b_1"][:])
                    for half in range(2):
                        base = p * PAIR + half * COLS
                        vr = min(max(V[t] - base, 0), COLS)
                        if vr == 0:
                            continue
                        nck = (vr + 127) // 128
                        outp = ppool.tile([128, COLS], F32, tag="uout")
                        for ck in range(nck):
                            m = min(vr - ck * 128, 128)
                            nc.tensor.matmul(
                                outp[0:m, ck * 128:(ck + 1) * 128],
                                h1[half * 64:(half + 1) * 64,
                                   ck * 128:ck * 128 + m],
                                w2[half * 64:(half + 1) * 64, :])
                        osb = opool.tile([128, COLS], F16, tag="osb")
                        nc.vector.tensor_tensor(osb[:, 0:nck * 128],
                                                outp[:, 0:nck * 128],
                                                b2t[:, 0:nck * 128], ALU.add)
                        dmae = nc.gpsimd if (p + half) % 2 else nc.sync
                        cf, pr = vr // 128, vr % 128
                        if cf > 0:
                            dmae.dma_start(
                                out=out_d[base:base + cf * 128, t, :]
                                .rearrange("(c p) d -> p c d", p=128),
                                in_=osb[:, 0:cf * 128]
                                .rearrange("p (c d) -> p c d", d=128))
                        if pr > 0:
                            dmae.dma_start(
                                out=out_d[base + cf * 128:base + vr, t, :],
                                in_=osb[0:pr, cf * 128:cf * 128 + 128])
    nc.compile()
    return nc


# ---------------------------------------------------------------------
# host-side numpy GRU+MLP for the remainder rows
def _host_rows(z_r, n_r, W_ih, W_hh, b_ih, b_hh, W1, b1, W2, b2):
    R = z_r.shape[0]
    out = np.zeros((R, MAXN, D), np.float32)
    if R == 0:
        return out

    def sig(v):
        return 1.0 / (1.0 + np.exp(-v))

    x = np.zeros_like(z_r)
    h = z_r.copy()
    for t in range(MAXN):
        gi = x @ W_ih.T + b_ih
        gh = h @ W_hh.T + b_hh
        r = sig(gi[:, :64] + gh[:, :64])
        u = sig(gi[:, 64:128] + gh[:, 64:128])
        c = np.tanh(gi[:, 128:] + r * gh[:, 128:])
        h = (1.0 - u) * c + u * h
        x = h
        out[:, t, :] = np.maximum(h @ W1 + b1, 0.0) @ W2 + b2
    out *= (np.arange(MAXN)[None, :, None] < n_r[:, None, None])
    return out


def kernel(z, n, W_ih, W_hh, b_ih, b_hh, W1, b1, W2, b2):
    global LAST_RESULTS
    z = np.asarray(z, np.float32)
    n = np.asarray(n, np.int32)
    W_ih = np.asarray(W_ih, np.float32)
    W_hh = np.asarray(W_hh, np.float32)
    b_ih = np.asarray(b_ih, np.float32)
    b_hh = np.asarray(b_hh, np.float32)
    W1 = np.asarray(W1, np.float32)
    b1 = np.asarray(b1, np.float32)
    W2 = np.asarray(W2, np.float32)
    b2 = np.asarray(b2, np.float32)

    # ---- partition rows across cores with identical n-multisets ------
    core_rows = [[] for _ in range(NCORES)]
    leftover = []
    n_core_vals = []
    for v in range(MAXN - 1, 0, -1):
        idx = np.where(n == v)[0]
        k = len(idx) // NCORES
        for c in range(NCORES):
            core_rows[c].append(idx[c * k:(c + 1) * k])
        leftover.append(idx[NCORES * k:])
        n_core_vals.append(np.full(k, v, np.int32))
    core_rows = [np.concatenate(cr) for cr in core_rows]
    leftover = np.concatenate(leftover)
    n_core = np.concatenate(n_core_vals) if n_core_vals else np.zeros(0, np.int32)

    B_real = len(core_rows[0])
    B_pad = max(((B_real + PAIR - 1) // PAIR) * PAIR, PAIR)
    n_sched = np.zeros(B_pad, np.int32)
    n_sched[:B_real] = n_core
    npairs = B_pad // PAIR
    S_pairs = tuple(int(n_sched[p * PAIR]) for p in range(npairs))
    V = tuple(int((n_sched > t).sum()) for t in range(MAXN))

    # ---- weights / schedule -> device constants ----------------------
    Wr = (W_ih[0:64] + W_hh[0:64]).T
    Wu = (W_ih[64:128] + W_hh[64:128]).T
    Wic = W_ih[128:192].T
    Whc = W_hh[128:192].T
    consts = {
        "w_r0": _blkdiag(W_hh[0:64].T),
        "w_u0": _blkdiag(W_hh[64:128].T),
        "w_hc0": _blkdiag(W_hh[128:192].T),
        "w_r": _blkdiag(Wr),
        "w_u": _blkdiag(Wu),
        "w_ic": _blkdiag(Wic),
        "w_hc": _blkdiag(Whc),
        "w_i": np.eye(128, dtype=np.float32),
        "w_h1": _blkdiag(W1),
    }
    consts = {k: v.astype(BF16) for k, v in consts.items()}
    consts["w2"] = np.vstack([W2, W2]).astype(BF16)
    consts["b2t"] = np.tile(np.tile(b2, 4)[None, :], (128, 1)).astype(np.float32)
    consts["b_r"] = np.tile(b_ih[0:64] + b_hh[0:64], 2)[:, None].astype(np.float32)
    consts["b_u"] = np.tile(b_ih[64:128] + b_hh[64:128], 2)[:, None].astype(np.float32)
    consts["b_ic"] = np.tile(b_ih[128:192], 2)[:, None].astype(np.float32)
    consts["b_hc"] = np.tile(b_hh[128:192], 2)[:, None].astype(np.float32)
    consts["b_1"] = np.tile(b1, 2)[:, None].astype(np.float32)

    key = (S_pairs, V, B_pad)
    if key not in _PROGRAM_CACHE:
        _PROGRAM_CACHE.clear()
        _PROGRAM_CACHE[key] = _build_program(S_pairs, V, B_pad)
    nc = _PROGRAM_CACHE[key]

    in_maps = []
    for c in range(NCORES):
        zmat = np.zeros((B_pad, H), np.float32)
        zmat[:B_real] = z[core_rows[c]]
        zr = zmat.reshape(npairs, 2, COLS, H)
        ztc = zr.transpose(1, 3, 0, 2).reshape(128, npairs * COLS)
        m = dict(consts)
        m["zt"] = np.ascontiguousarray(ztc).astype(BF16)
        in_maps.append(m)

    res = run_bass_kernel_spmd(nc, in_maps, list(range(NCORES)))
    LAST_RESULTS = res

    # ---- gather ------------------------------------------------------
    x_out = np.zeros((B, MAXN, D), np.float32)
    for c in range(NCORES):
        x_out[core_rows[c]] = res.results[c]["out"][:B_real].astype(np.float32)
    if len(leftover):
        x_out[leftover] = _host_rows(z[leftover], n[leftover], W_ih, W_hh,
                                     b_ih, b_hh, W1, b1, W2, b2)
    mask = np.arange(MAXN)[None, :] < n[:, None]
    return x_out, mask
